# revision 7
# baseline (speedup 1.0000x reference)
"""Fused QKV + multi-head attention kernel for Trainium2 (Bass/Tile), 8-core SPMD.

Problem: x[4, 2048, 1024] -> qkv = x @ W_qkv + b_qkv -> 16-head attention -> out[4, 2048, 1024].

Sharding (DP x TP): core c handles batch c//2 and head-group c%2 (8 of 16 heads),
so each core runs the qkv projection for its batch restricted to its heads'
columns of W_qkv, plus full attention for its 8 heads. No cross-core comm.

Per-core kernel design (bf16 matmuls, fp32 accumulation):
 - the host pre-transposes x during sharding, so the device loads xT[k, tok]
   directly with a casting DMA (f32->bf16); token-block-major DMA order lets
   the first k-projection chains start before the full x is resident.
 - qk projection computes qkv^T directly: lhsT = W tile, rhs = xT. Heads are
   processed in pairs: head 2p on SBUF partitions 0-63, head 2p+1 on 64-127,
   making the K=64 score matmuls row-tile onto independent 64-row PE tiles.
 - v projection computes v in [tok, feat] orientation with a ones-column per
   head so the attention A@V matmul also produces the softmax denominator.
 - softmax exp is the scalar-engine bottleneck (1 elem/lane/cycle, +352cyc
   per instruction, PSUM-read free dim capped at 1024 by the 2-bank tiles).
   To balance engines, each query-group's 16 key-tiles split two ways:
     * jj 9-15 "direct": ACT exp reads scores straight from PSUM [128,1024].
     * jj 0-8 "copied": DVE copies scores PSUM->SBUF bf16; the exp for those
       9 blocks runs one window later as two long-free ACT instructions
       ([128,5120] + [128,4096]) amortizing the per-instruction overhead.
   The A@V accumulation for a group therefore wraps across windows: start=
   jj9 (direct, this window), stop=jj8 (copied, next window). po double-
   buffering hands the two PSUM banks over mid-window.
 - output normalization (1/den broadcast multiply + v-bias) runs on GpSimd
   to keep DVE free for the score copies; the denominator reciprocal is
   computed 128-lane-wide via a DRAM bounce and stride-0 broadcast DMA.
"""

import sys

sys.path.insert(0, "/opt/trn_rl_repo")

import numpy as np
import ml_dtypes

T = 2048
D = 1024
NH_LOCAL = 8  # heads per core
HS = 64
WCOLS = NH_LOCAL * 3 * HS  # 1536
VCOLS = NH_LOCAL * HS  # 512
KT = D // 128  # 8 contraction tiles
PAIRS = NH_LOCAL // 2  # 4
IG = T // 512  # 4 query groups
JT = T // 128  # 16 key tiles
NCOPY = 12  # key tiles 0..11 take the copied (big-exp) path
NBIGA = 6  # copied blocks per first big exp ([128, 6144])

_CACHE: dict = {}


def _emit(tc, x_d, w_d, bqk_d, bv_d, out_d):
    import concourse.bass as bass
    from concourse import mybir
    from contextlib import ExitStack

    nc = tc.nc
    f32 = mybir.dt.float32
    bf16 = mybir.dt.bfloat16
    Exp = mybir.ActivationFunctionType.Exp

    ctx = ExitStack()
    pers = ctx.enter_context(tc.tile_pool(name="pers", bufs=1))
    qk_pool = ctx.enter_context(tc.tile_pool(name="qk", bufs=3))
    stage_pool = ctx.enter_context(tc.tile_pool(name="stage", bufs=2))
    bige_pool = ctx.enter_context(tc.tile_pool(name="bige", bufs=2))
    e_pool = ctx.enter_context(tc.tile_pool(name="epool", bufs=3))
    o_pool = ctx.enter_context(tc.tile_pool(name="opool", bufs=3))
    sm_pool = ctx.enter_context(tc.tile_pool(name="smpool", bufs=4))
    ot_pool = ctx.enter_context(tc.tile_pool(name="otpool", bufs=4))
    dr_pool = ctx.enter_context(tc.tile_pool(name="drpool", bufs=4, space="DRAM"))
    # PSUM: 8 banks = ps 2 (proj) + pss 2x2 (score staging) + po 2 (AV accum).
    ps_pool = ctx.enter_context(tc.tile_pool(name="pspool", bufs=2, space="PSUM"))
    pss_pool = ctx.enter_context(tc.tile_pool(name="psspool", bufs=2, space="PSUM"))
    po_pool = ctx.enter_context(tc.tile_pool(name="popool", bufs=2, space="PSUM"))

    # ---- DMA order: biases, pair-0 qk weights, then x token-block-major
    #      (first k chains start after ~1.5MB), v weights mid-stream, and the
    #      pairs-1-3 qk weights last (not needed until window 2).
    xt_sb = pers.tile([128, KT, T], bf16)  # xT[k, tok] per k-tile
    w_sb = pers.tile([128, KT, WCOLS], bf16)
    bqk_sb = pers.tile([128, PAIRS, 2], f32)  # [part, pair, q/k] per-partition bias
    nc.sync.dma_start(bqk_sb, bqk_d)
    bv_pp = pers.tile([64, NH_LOCAL], f32)  # v-bias, per-partition layout [d, head]
    nc.sync.dma_start(bv_pp, bv_d.rearrange("(h d) -> d h", d=HS))
    for kk in range(KT):
        nc.sync.dma_start(w_sb[:, kk, 0:256], w_d[kk * 128 : (kk + 1) * 128, 0:256])
    for g in range(IG):
        for kk in range(KT):
            nc.sync.dma_start(
                xt_sb[:, kk, g * 512 : (g + 1) * 512],
                x_d[kk * 128 : (kk + 1) * 128, g * 512 : (g + 1) * 512],
            )
        if g == 1:
            for kk in range(KT):
                nc.sync.dma_start(
                    w_sb[:, kk, 1024:1536], w_d[kk * 128 : (kk + 1) * 128, 1024:1536]
                )
    for kk in range(KT):
        nc.sync.dma_start(
            w_sb[:, kk, 256:1024], w_d[kk * 128 : (kk + 1) * 128, 256:1024]
        )

    # ---- v ones-columns ----
    v_sb = pers.tile([128, JT, NH_LOCAL, HS + 1], bf16)
    nc.vector.memset(v_sb[:, :, :, HS : HS + 1], 1.0)

    # w is host-permuted: cols [(p*2+qk)*128 : +128] = paired q/k block for pair p,
    # cols [1024:1536] = v columns in head order.
    def qk_chain(p, g, qk):
        ps = ps_pool.tile([128, 512], f32, tag="ps", name="psqk")
        c0 = (p * 2 + qk) * 128
        for kk in range(KT):
            nc.tensor.matmul(
                ps,
                w_sb[:, kk, c0 : c0 + 128],
                xt_sb[:, kk, g * 512 : (g + 1) * 512],
                start=(kk == 0),
                stop=(kk == KT - 1),
            )
        nc.vector.tensor_scalar_add(
            qk_tiles[p][:, qk, g * 512 : (g + 1) * 512], ps, bqk_sb[:, p, qk : qk + 1]
        )

    def v_chain(tt):
        ps = ps_pool.tile([128, 512], f32, tag="ps", name="psv")
        for kk in range(KT):
            nc.tensor.matmul(
                ps,
                xt_sb[:, kk, tt * 128 : (tt + 1) * 128],
                w_sb[:, kk, 1024:1536],
                start=(kk == 0),
                stop=(kk == KT - 1),
            )
        nc.vector.tensor_copy(
            v_sb[:, tt, :, 0:HS], ps.rearrange("p (h c) -> p h c", c=HS)
        )

    qk_tiles = [
        qk_pool.tile([128, 2, T], bf16, tag="qkt", name=f"qkt{_p}") for _p in range(PAIRS)
    ]

    def scores(p, ig, jj):
        """scores^T for both heads of pair p, key tile jj, query group ig."""
        qk_t = qk_tiles[p]
        ps = pss_pool.tile([128, 1024], f32, tag="pss")
        for h in range(2):
            base = 64 * h
            nc.tensor.matmul(
                ps[:, h * 512 : (h + 1) * 512],
                qk_t[base : base + 64, 1, jj * 128 : (jj + 1) * 128],
                qk_t[base : base + 64, 0, ig * 512 : (ig + 1) * 512],
                start=True,
                stop=True,
            )
        return ps

    def av(p, po, e_ap, jj, start, stop):
        """po[h] += [v|1]^T @ E for both heads; e_ap is [128, 1024] (h0|h1)."""
        for h in range(2):
            nc.tensor.matmul(
                po[h],
                v_sb[:, jj, 2 * p + h, :],
                e_ap[:, h * 512 : (h + 1) * 512],
                start=start,
                stop=stop,
            )

    def normalize(p, ig, po):
        """den-normalize po, add v-bias, store out^T (gpsimd does the muls)."""
        o_t = [o_pool.tile([65, 512], f32, tag="o", name=f"ot{_h}") for _h in range(2)]
        rcd_t = dr_pool.tile([2, 512], f32, tag="rcd")
        rct = sm_pool.tile([128, 8], f32, tag="rct")
        for h in range(2):
            nc.vector.tensor_copy(o_t[h], po[h])
            nc.sync.dma_start(rct[:, h * 4 : (h + 1) * 4], o_t[h][64:65, :])
        rcp = sm_pool.tile([128, 8], f32, tag="rcp")
        nc.vector.reciprocal(rcp, rct)
        for h in range(2):
            nc.sync.dma_start(rcd_t[h], rcp[:, h * 4 : (h + 1) * 4])
        for h in range(2):
            den_bc = sm_pool.tile([64, 512], f32, tag="denbc", name=f"dbc{h}")
            rcd_h = rcd_t[h]
            bc_src = bass.AP(
                tensor=rcd_h.tensor,
                offset=rcd_h.offset,
                ap=[[0, 64]] + list(rcd_h.ap),
            )
            nc.gpsimd.dma_start(den_bc, bc_src)
            ot = ot_pool.tile([64, 512], f32, tag="ot")
            nc.gpsimd.tensor_mul(ot, o_t[h][0:64, :], den_bc)
            nc.gpsimd.tensor_scalar_add(ot, ot, bv_pp[:, 2 * p + h : 2 * p + h + 1])
            nc.sync.dma_start(
                out_d[(2 * p + h) * HS : (2 * p + h + 1) * HS, ig * 512 : (ig + 1) * 512],
                ot,
            )

    # ---------------- window pipeline ----------------
    # window w = (p, ig); carries over: copied-score stage + po of window w-1.
    windows = [(p, ig) for p in range(PAIRS) for ig in range(IG)]

    # PE filler chains per window (projection work woven into PE slack).
    # pair 0: k(g0..3)+q(g0) run in the prologue; v tiles 12-15 must precede
    # this window's direct AVs, v 0-11 the copied AVs that run early in w1.
    fillers = {w: [] for w in range(len(windows) + 1)}

    def F(w, fn, *a):
        import functools

        fillers[w].append(functools.partial(fn, *a))

    for tt in range(NCOPY, JT):
        F(0, v_chain, tt)
    F(0, qk_chain, 0, 1, 0)
    for tt in range(0, 6):
        F(0, v_chain, tt)
    for tt in range(6, NCOPY):
        F(1, v_chain, tt)
    F(1, qk_chain, 0, 2, 0)
    F(2, qk_chain, 0, 3, 0)
    for p in range(1, PAIRS):
        base = 4 * (p - 1)
        # next pair's k chains late in the previous pair, its q(g0) last
        F(base + 2, qk_chain, p, 0, 1)
        F(base + 2, qk_chain, p, 1, 1)
        F(base + 3, qk_chain, p, 2, 1)
        F(base + 3, qk_chain, p, 3, 1)
        F(base + 3, qk_chain, p, 0, 0)
        # own later q chains woven into own windows
        F(4 * p + 0, qk_chain, p, 1, 0)
        F(4 * p + 1, qk_chain, p, 2, 0)
        F(4 * p + 2, qk_chain, p, 3, 0)

    # prologue: pair-0 k chains for all query groups + q(g0)
    for g in range(IG):
        qk_chain(0, g, 1)
    qk_chain(0, 0, 0)

    prev = None  # (p, ig, po, stage)
    for w, (p, ig) in enumerate(windows):
        fill = list(fillers[w])

        def pop_fill(n=1):
            for _ in range(n):
                if fill:
                    fill.pop(0)()

        stage = stage_pool.tile([128, NCOPY * 1024], bf16, tag="stage")
        po = [
            po_pool.tile([65, 512], f32, tag="po", name=f"po{_h}") for _h in range(2)
        ]

        # -- copied-path scores + DVE copies for this window, interleaved with
        #    the big exps + AVs that finish the PREVIOUS window's group. The
        #    AVc bursts are spread between score matmuls (4 MMs per slot) so
        #    the PE's FIFO never blocks score production for long.
        bige_a = bige_b = None
        if prev is not None:
            pp, pig, ppo, pstage = prev
            bige_a = bige_pool.tile([128, NBIGA * 1024], bf16, tag="bige")
            nc.scalar.activation(bige_a, pstage[:, : NBIGA * 1024], Exp, scale=0.125)
        for jj in range(NCOPY):
            ps = scores(p, ig, jj)
            nc.vector.tensor_copy(stage[:, jj * 1024 : (jj + 1) * 1024], ps)
            pop_fill()
            if prev is not None:
                if jj == 5:
                    bige_b = bige_pool.tile(
                        [128, (NCOPY - NBIGA) * 1024], bf16, tag="bige"
                    )
                    nc.scalar.activation(
                        bige_b, pstage[:, NBIGA * 1024 :], Exp, scale=0.125
                    )
                if 6 <= jj < 9:
                    for b in range(2 * (jj - 6), 2 * (jj - 5)):
                        av(pp, ppo, bige_a[:, b * 1024 : (b + 1) * 1024], b,
                           start=False, stop=False)
                if 9 <= jj < 12:
                    for b in range(NBIGA + 2 * (jj - 9), NBIGA + 2 * (jj - 8)):
                        av(pp, ppo,
                           bige_b[:, (b - NBIGA) * 1024 : (b - NBIGA + 1) * 1024],
                           b, start=False, stop=(b == NCOPY - 1))
        if prev is not None:
            normalize(pp, pig, ppo)

        # -- direct path: jj 12..15, exp straight from PSUM, AV immediately
        for jj in range(NCOPY, JT):
            ps = scores(p, ig, jj)
            e_t = e_pool.tile([128, 1024], bf16, tag="e")
            nc.scalar.activation(e_t, ps, Exp, scale=0.125)
            av(p, po, e_t, jj, start=(jj == NCOPY), stop=False)
            pop_fill()
        while fill:
            fill.pop(0)()

        prev = (p, ig, po, stage)

    # ---- drain the last window's copied path ----
    pp, pig, ppo, pstage = prev
    bige_a = bige_pool.tile([128, NBIGA * 1024], bf16, tag="bige")
    nc.scalar.activation(bige_a, pstage[:, : NBIGA * 1024], Exp, scale=0.125)
    for b in range(NBIGA):
        av(pp, ppo, bige_a[:, b * 1024 : (b + 1) * 1024], b, start=False, stop=False)
    bige_b = bige_pool.tile([128, (NCOPY - NBIGA) * 1024], bf16, tag="bige")
    nc.scalar.activation(bige_b, pstage[:, NBIGA * 1024 :], Exp, scale=0.125)
    for b in range(NBIGA, NCOPY):
        av(pp, ppo, bige_b[:, (b - NBIGA) * 1024 : (b - NBIGA + 1) * 1024], b,
           start=False, stop=(b == NCOPY - 1))
    normalize(pp, pig, ppo)
    ctx.close()


def _build():
    import concourse.tile as tile
    from concourse import bacc, mybir

    f32 = mybir.dt.float32
    nc = bacc.Bacc("TRN2", target_bir_lowering=False, debug=False, num_devices=8)
    x_d = nc.dram_tensor("x", [D, T], mybir.dt.bfloat16, kind="ExternalInput").ap()
    w_d = nc.dram_tensor("w", [D, WCOLS], mybir.dt.bfloat16, kind="ExternalInput").ap()
    bqk_d = nc.dram_tensor("bqk", [128, PAIRS, 2], f32, kind="ExternalInput").ap()
    bv_d = nc.dram_tensor("bv", [VCOLS], f32, kind="ExternalInput").ap()
    out_d = nc.dram_tensor("out", [VCOLS, T], f32, kind="ExternalOutput").ap()
    with tile.TileContext(nc) as tc:
        _emit(tc, x_d, w_d, bqk_d, bv_d, out_d)
    nc.compile()
    return nc


def get_nc():
    if "nc" not in _CACHE:
        _CACHE["nc"] = _build()
    return _CACHE["nc"]


def make_in_maps(x, W_qkv, b_qkv):
    """Shard full inputs into 8 per-core input maps."""
    x = np.asarray(x, dtype=np.float32)
    W_qkv = np.asarray(W_qkv, dtype=np.float32)
    b_qkv = np.asarray(b_qkv, dtype=np.float32)
    in_maps = []
    for c in range(8):
        b, half = divmod(c, 2)
        w_c = W_qkv[:, half * WCOLS : (half + 1) * WCOLS]
        b_c = b_qkv[half * WCOLS : (half + 1) * WCOLS]
        # permute columns: paired q/k blocks first, then v cols in head order
        w3 = w_c.reshape(D, NH_LOCAL, 3, HS)
        blocks = []
        for p in range(PAIRS):
            for qk in range(2):
                blocks.append(w3[:, 2 * p, qk, :])
                blocks.append(w3[:, 2 * p + 1, qk, :])
        for h in range(NH_LOCAL):
            blocks.append(w3[:, h, 2, :])
        w_c = np.concatenate(blocks, axis=1).astype(ml_dtypes.bfloat16)
        # per-partition qk bias: partitions 0-63 <- head 2p, 64-127 <- head 2p+1
        bqk = np.zeros((128, PAIRS, 2), dtype=np.float32)
        for p in range(PAIRS):
            for qk in range(2):
                bqk[0:64, p, qk] = b_c[(2 * p) * 192 + qk * 64 : (2 * p) * 192 + (qk + 1) * 64]
                bqk[64:128, p, qk] = b_c[(2 * p + 1) * 192 + qk * 64 : (2 * p + 1) * 192 + (qk + 1) * 64]
        bv = np.ascontiguousarray(
            b_c.reshape(NH_LOCAL, 3, HS)[:, 2, :].reshape(VCOLS)
        )
        in_maps.append(
            {
                "x": np.ascontiguousarray(x[b].T).astype(ml_dtypes.bfloat16),
                "w": w_c,
                "bqk": bqk,
                "bv": bv,
            }
        )
    return in_maps


def assemble_output(results):
    out = np.zeros((4, T, D), dtype=np.float32)
    for c in range(8):
        b, half = divmod(c, 2)
        out[b, :, half * VCOLS : (half + 1) * VCOLS] = results[c]["out"].T
    return out


def kernel(x, W_qkv, b_qkv):
    from concourse.bass_utils import run_bass_kernel_spmd

    nc = get_nc()
    in_maps = make_in_maps(x, W_qkv, b_qkv)
    res = run_bass_kernel_spmd(nc, in_maps, core_ids=list(range(8)))
    return assemble_output(res.results)


if __name__ == "__main__":
    xs = np.random.randn(4, T, D).astype(np.float32)
    Ws = (np.random.randn(D, 3 * D) / 32.0).astype(np.float32)
    bs = (np.random.randn(3 * D) * 0.02).astype(np.float32)
    o = kernel(xs, Ws, bs)
    print(o.shape, o.dtype)


# revision 13
# speedup vs baseline: 1.0709x; 1.0709x over previous
"""Fused QKV + multi-head attention kernel for Trainium2 (Bass/Tile), 8-core SPMD.

Problem: x[4, 2048, 1024] -> qkv = x @ W_qkv + b_qkv -> 16-head attention -> out[4, 2048, 1024].

Sharding (DP x TP): core c handles batch c//2 and head-group c%2 (8 of 16 heads),
so each core runs the qkv projection for its batch restricted to its heads'
columns of W_qkv, plus full attention for its 8 heads. No cross-core comm.

Per-core kernel design (bf16 matmuls, fp32 accumulation):
 - the host pre-transposes x during sharding, so the device loads xT[k, tok]
   directly with a casting DMA (f32->bf16); token-block-major DMA order lets
   the first k-projection chains start before the full x is resident.
 - qk projection computes qkv^T directly: lhsT = W tile, rhs = xT. Heads are
   processed in pairs: head 2p on SBUF partitions 0-63, head 2p+1 on 64-127,
   making the K=64 score matmuls row-tile onto independent 64-row PE tiles.
 - v projection computes v in [tok, feat] orientation with a ones-column per
   head so the attention A@V matmul also produces the softmax denominator.
 - softmax exp is the scalar-engine bottleneck (1 elem/lane/cycle, +352cyc
   per instruction, PSUM-read free dim capped at 1024 by the 2-bank tiles).
   To balance engines, each query-group's 16 key-tiles split two ways:
     * jj 9-15 "direct": ACT exp reads scores straight from PSUM [128,1024].
     * jj 0-8 "copied": DVE copies scores PSUM->SBUF bf16; the exp for those
       9 blocks runs one window later as two long-free ACT instructions
       ([128,5120] + [128,4096]) amortizing the per-instruction overhead.
   The A@V accumulation for a group therefore wraps across windows: start=
   jj9 (direct, this window), stop=jj8 (copied, next window). po double-
   buffering hands the two PSUM banks over mid-window.
 - output normalization (1/den broadcast multiply + v-bias) runs on GpSimd
   to keep DVE free for the score copies; the denominator reciprocal is
   computed 128-lane-wide via a DRAM bounce and stride-0 broadcast DMA.
"""

import sys

sys.path.insert(0, "/opt/trn_rl_repo")

import numpy as np
import ml_dtypes

T = 2048
D = 1024
NH_LOCAL = 8  # heads per core
HS = 64
WCOLS = NH_LOCAL * 3 * HS  # 1536
VCOLS = NH_LOCAL * HS  # 512
KT = D // 128  # 8 contraction tiles
PAIRS = NH_LOCAL // 2  # 4
IG = T // 512  # 4 query groups
JT = T // 128  # 16 key tiles
NCOPY = 11  # key tiles 0..10 take the copied (big-exp) path
NBIGA = 6  # copied blocks per first big exp ([128, 6144])

_CACHE: dict = {}


def _emit(tc, x_d, w_d, bqk_d, bv_d, out_d):
    import concourse.bass as bass
    from concourse import mybir
    from contextlib import ExitStack

    nc = tc.nc
    f32 = mybir.dt.float32
    bf16 = mybir.dt.bfloat16
    Exp = mybir.ActivationFunctionType.Exp

    ctx = ExitStack()
    pers = ctx.enter_context(tc.tile_pool(name="pers", bufs=1))
    qk_pool = ctx.enter_context(tc.tile_pool(name="qk", bufs=3))
    stage_pool = ctx.enter_context(tc.tile_pool(name="stage", bufs=2))
    bige_pool = ctx.enter_context(tc.tile_pool(name="bige", bufs=2))
    e_pool = ctx.enter_context(tc.tile_pool(name="epool", bufs=3))
    o_pool = ctx.enter_context(tc.tile_pool(name="opool", bufs=3))
    sm_pool = ctx.enter_context(tc.tile_pool(name="smpool", bufs=4))
    ot_pool = ctx.enter_context(tc.tile_pool(name="otpool", bufs=4))
    dr_pool = ctx.enter_context(tc.tile_pool(name="drpool", bufs=4, space="DRAM"))
    # PSUM: 8 banks = ps 2 (proj) + pss 2x2 (score staging) + po 2 (AV accum).
    ps_pool = ctx.enter_context(tc.tile_pool(name="pspool", bufs=2, space="PSUM"))
    pss_pool = ctx.enter_context(tc.tile_pool(name="psspool", bufs=2, space="PSUM"))
    po_pool = ctx.enter_context(tc.tile_pool(name="popool", bufs=2, space="PSUM"))

    # ---- DMA order: biases, pair-0 qk weights, then x token-block-major
    #      (first k chains start after ~1.5MB), v weights mid-stream, and the
    #      pairs-1-3 qk weights last (not needed until window 2).
    xt_sb = pers.tile([128, KT, T], bf16)  # xT[k, tok] per k-tile
    w_sb = pers.tile([128, KT, WCOLS], bf16)
    bqk_sb = pers.tile([128, PAIRS, 2], f32)  # [part, pair, q/k] per-partition bias
    nc.sync.dma_start(bqk_sb, bqk_d)
    bv_pp = pers.tile([64, NH_LOCAL], f32)  # v-bias, per-partition layout [d, head]
    nc.sync.dma_start(bv_pp, bv_d.rearrange("(h d) -> d h", d=HS))
    for kk in range(KT):
        nc.sync.dma_start(w_sb[:, kk, 0:256], w_d[kk * 128 : (kk + 1) * 128, 0:256])
    # x split across two otherwise-idle queues; v/rest-of-W on a third
    for g in range(IG):
        for kk in range(KT):
            q = nc.sync if kk % 2 == 0 else nc.scalar
            q.dma_start(
                xt_sb[:, kk, g * 512 : (g + 1) * 512],
                x_d[kk * 128 : (kk + 1) * 128, g * 512 : (g + 1) * 512],
            )
    for kk in range(KT):
        nc.gpsimd.dma_start(
            w_sb[:, kk, 1024:1536], w_d[kk * 128 : (kk + 1) * 128, 1024:1536]
        )
    for kk in range(KT):
        nc.gpsimd.dma_start(
            w_sb[:, kk, 256:1024], w_d[kk * 128 : (kk + 1) * 128, 256:1024]
        )

    # ---- v ones-columns ----
    v_sb = pers.tile([128, JT, NH_LOCAL, HS + 1], bf16)
    nc.vector.memset(v_sb[:, :, :, HS : HS + 1], 1.0)

    # w is host-permuted: cols [(p*2+qk)*128 : +128] = paired q/k block for pair p,
    # cols [1024:1536] = v columns in head order.
    def qk_chain(p, g, qk):
        ps = ps_pool.tile([128, 512], f32, tag="ps", name="psqk")
        c0 = (p * 2 + qk) * 128
        for kk in range(KT):
            nc.tensor.matmul(
                ps,
                w_sb[:, kk, c0 : c0 + 128],
                xt_sb[:, kk, g * 512 : (g + 1) * 512],
                start=(kk == 0),
                stop=(kk == KT - 1),
            )
        nc.vector.tensor_scalar_add(
            qk_tiles[p][:, qk, g * 512 : (g + 1) * 512], ps, bqk_sb[:, p, qk : qk + 1]
        )

    def v_chain(tt):
        ps = ps_pool.tile([128, 512], f32, tag="ps", name="psv")
        for kk in range(KT):
            nc.tensor.matmul(
                ps,
                xt_sb[:, kk, tt * 128 : (tt + 1) * 128],
                w_sb[:, kk, 1024:1536],
                start=(kk == 0),
                stop=(kk == KT - 1),
            )
        nc.vector.tensor_copy(
            v_sb[:, tt, :, 0:HS], ps.rearrange("p (h c) -> p h c", c=HS)
        )

    qk_tiles = [
        qk_pool.tile([128, 2, T], bf16, tag="qkt", name=f"qkt{_p}") for _p in range(PAIRS)
    ]

    def scores(p, ig, jj):
        """scores^T for both heads of pair p, key tile jj, query group ig."""
        qk_t = qk_tiles[p]
        ps = pss_pool.tile([128, 1024], f32, tag="pss")
        for h in range(2):
            base = 64 * h
            nc.tensor.matmul(
                ps[:, h * 512 : (h + 1) * 512],
                qk_t[base : base + 64, 1, jj * 128 : (jj + 1) * 128],
                qk_t[base : base + 64, 0, ig * 512 : (ig + 1) * 512],
                start=True,
                stop=True,
            )
        return ps

    def av(p, po, e_ap, jj, start, stop):
        """po[h] += [v|1]^T @ E for both heads; e_ap is [128, 1024] (h0|h1)."""
        for h in range(2):
            nc.tensor.matmul(
                po[h],
                v_sb[:, jj, 2 * p + h, :],
                e_ap[:, h * 512 : (h + 1) * 512],
                start=start,
                stop=stop,
            )

    def normalize(p, ig, po):
        """den-normalize po, add v-bias, store out^T (gpsimd does the muls)."""
        o_t = [o_pool.tile([65, 512], f32, tag="o", name=f"ot{_h}") for _h in range(2)]
        rcd_t = dr_pool.tile([2, 512], f32, tag="rcd")
        rct = sm_pool.tile([128, 8], f32, tag="rct")
        for h in range(2):
            nc.vector.tensor_copy(o_t[h], po[h])
            nc.sync.dma_start(rct[:, h * 4 : (h + 1) * 4], o_t[h][64:65, :])
        rcp = sm_pool.tile([128, 8], f32, tag="rcp")
        nc.vector.reciprocal(rcp, rct)
        for h in range(2):
            nc.sync.dma_start(rcd_t[h], rcp[:, h * 4 : (h + 1) * 4])
        for h in range(2):
            den_bc = sm_pool.tile([64, 512], f32, tag="denbc", name=f"dbc{h}")
            rcd_h = rcd_t[h]
            bc_src = bass.AP(
                tensor=rcd_h.tensor,
                offset=rcd_h.offset,
                ap=[[0, 64]] + list(rcd_h.ap),
            )
            nc.gpsimd.dma_start(den_bc, bc_src)
            ot = ot_pool.tile([64, 512], f32, tag="ot")
            # gpsimd tensor_tensor is fast; its tensor_scalar is ~7.5us (slow
            # ucode path) so the per-partition bias add stays on DVE.
            nc.gpsimd.tensor_mul(ot, o_t[h][0:64, :], den_bc)
            nc.vector.tensor_scalar_add(ot, ot, bv_pp[:, 2 * p + h : 2 * p + h + 1])
            nc.sync.dma_start(
                out_d[(2 * p + h) * HS : (2 * p + h + 1) * HS, ig * 512 : (ig + 1) * 512],
                ot,
            )

    # ---------------- window pipeline ----------------
    # window w = (p, ig); carries over: copied-score stage + po of window w-1.
    windows = [(p, ig) for p in range(PAIRS) for ig in range(IG)]

    # PE filler chains per window (projection work woven into PE slack).
    # pair 0: k(g0..3)+q(g0) run in the prologue; v tiles 12-15 must precede
    # this window's direct AVs, v 0-11 the copied AVs that run early in w1.
    fillers = {w: [] for w in range(len(windows) + 1)}

    def F(w, fn, *a):
        import functools

        fillers[w].append(functools.partial(fn, *a))

    # window 0 runs all-direct (no previous group to overlap), so every v
    # tile must be ready in jj order for its AV; the fillers pop in order.
    for tt in range(JT):
        F(0, v_chain, tt)
    F(0, qk_chain, 0, 1, 0)
    F(1, qk_chain, 0, 2, 0)
    F(2, qk_chain, 0, 3, 0)
    for p in range(1, PAIRS):
        base = 4 * (p - 1)
        # next pair's k chains late in the previous pair, its q(g0) last
        F(base + 2, qk_chain, p, 0, 1)
        F(base + 2, qk_chain, p, 1, 1)
        F(base + 3, qk_chain, p, 2, 1)
        F(base + 3, qk_chain, p, 3, 1)
        F(base + 3, qk_chain, p, 0, 0)
        # own later q chains woven into own windows
        F(4 * p + 0, qk_chain, p, 1, 0)
        F(4 * p + 1, qk_chain, p, 2, 0)
        F(4 * p + 2, qk_chain, p, 3, 0)

    # prologue: pair-0 k chains for all query groups + q(g0)
    for g in range(IG):
        qk_chain(0, g, 1)
    qk_chain(0, 0, 0)

    prev = None  # (p, ig, po, stage)
    for w, (p, ig) in enumerate(windows):
        fill = list(fillers[w])

        def pop_fill(n=1):
            for _ in range(n):
                if fill:
                    fill.pop(0)()

        po = [
            po_pool.tile([65, 512], f32, tag="po", name=f"po{_h}") for _h in range(2)
        ]

        if w == 0:
            # all-direct first window: ACT has no carried-over big exps yet,
            # and the v chains must weave in jj order ahead of each AV.
            for jj in range(JT):
                ps = scores(p, ig, jj)
                e_t = e_pool.tile([128, 1024], bf16, tag="e")
                nc.scalar.activation(e_t, ps, Exp, scale=0.125)
                pop_fill()
                av(p, po, e_t, jj, start=(jj == 0), stop=(jj == JT - 1))
            while fill:
                fill.pop(0)()
            normalize(p, ig, po)
            prev = None
            continue

        stage = stage_pool.tile([128, NCOPY * 1024], bf16, tag="stage")

        # -- copied-path scores + DVE copies for this window, interleaved with
        #    the big exps + AVs that finish the PREVIOUS window's group. The
        #    AVc bursts are spread between score matmuls so the PE's FIFO
        #    never blocks score production for long.
        bige_a = bige_b = None
        if prev is not None:
            pp, pig, ppo, pstage = prev
            bige_a = bige_pool.tile([128, NBIGA * 1024], bf16, tag="bige")
            nc.scalar.activation(bige_a, pstage[:, : NBIGA * 1024], Exp, scale=0.125)
        for jj in range(NCOPY):
            ps = scores(p, ig, jj)
            nc.vector.tensor_copy(stage[:, jj * 1024 : (jj + 1) * 1024], ps)
            pop_fill()
            if prev is not None:
                if jj == 5:
                    bige_b = bige_pool.tile(
                        [128, (NCOPY - NBIGA) * 1024], bf16, tag="bige"
                    )
                    nc.scalar.activation(
                        bige_b, pstage[:, NBIGA * 1024 :], Exp, scale=0.125
                    )
                if 6 <= jj < 9:
                    bs = [2 * (jj - 6), 2 * (jj - 6) + 1]
                elif jj == 9:
                    bs = [6, 7]
                elif jj == 10:
                    bs = [8, 9, 10]
                else:
                    bs = []
                for b in bs:
                    if b < NBIGA:
                        e_ap = bige_a[:, b * 1024 : (b + 1) * 1024]
                    else:
                        e_ap = bige_b[:, (b - NBIGA) * 1024 : (b - NBIGA + 1) * 1024]
                    av(pp, ppo, e_ap, b, start=False, stop=(b == NCOPY - 1))
        if prev is not None:
            normalize(pp, pig, ppo)

        # -- direct path: jj 11..15, exp straight from PSUM, AV immediately
        for jj in range(NCOPY, JT):
            ps = scores(p, ig, jj)
            e_t = e_pool.tile([128, 1024], bf16, tag="e")
            nc.scalar.activation(e_t, ps, Exp, scale=0.125)
            av(p, po, e_t, jj, start=(jj == NCOPY), stop=False)
            pop_fill()
        while fill:
            fill.pop(0)()

        prev = (p, ig, po, stage)

    # ---- drain the last window's copied path ----
    pp, pig, ppo, pstage = prev
    bige_a = bige_pool.tile([128, NBIGA * 1024], bf16, tag="bige")
    nc.scalar.activation(bige_a, pstage[:, : NBIGA * 1024], Exp, scale=0.125)
    for b in range(NBIGA):
        av(pp, ppo, bige_a[:, b * 1024 : (b + 1) * 1024], b, start=False, stop=False)
    bige_b = bige_pool.tile([128, (NCOPY - NBIGA) * 1024], bf16, tag="bige")
    nc.scalar.activation(bige_b, pstage[:, NBIGA * 1024 :], Exp, scale=0.125)
    for b in range(NBIGA, NCOPY):
        av(pp, ppo, bige_b[:, (b - NBIGA) * 1024 : (b - NBIGA + 1) * 1024], b,
           start=False, stop=(b == NCOPY - 1))
    normalize(pp, pig, ppo)
    ctx.close()


def _build():
    import concourse.tile as tile
    from concourse import bacc, mybir

    f32 = mybir.dt.float32
    nc = bacc.Bacc("TRN2", target_bir_lowering=False, debug=False, num_devices=8)
    x_d = nc.dram_tensor("x", [D, T], mybir.dt.bfloat16, kind="ExternalInput").ap()
    w_d = nc.dram_tensor("w", [D, WCOLS], mybir.dt.bfloat16, kind="ExternalInput").ap()
    bqk_d = nc.dram_tensor("bqk", [128, PAIRS, 2], f32, kind="ExternalInput").ap()
    bv_d = nc.dram_tensor("bv", [VCOLS], f32, kind="ExternalInput").ap()
    out_d = nc.dram_tensor("out", [VCOLS, T], f32, kind="ExternalOutput").ap()
    with tile.TileContext(nc) as tc:
        _emit(tc, x_d, w_d, bqk_d, bv_d, out_d)
    nc.compile()
    return nc


def get_nc():
    if "nc" not in _CACHE:
        _CACHE["nc"] = _build()
    return _CACHE["nc"]


def make_in_maps(x, W_qkv, b_qkv):
    """Shard full inputs into 8 per-core input maps."""
    x = np.asarray(x, dtype=np.float32)
    W_qkv = np.asarray(W_qkv, dtype=np.float32)
    b_qkv = np.asarray(b_qkv, dtype=np.float32)
    in_maps = []
    for c in range(8):
        b, half = divmod(c, 2)
        w_c = W_qkv[:, half * WCOLS : (half + 1) * WCOLS]
        b_c = b_qkv[half * WCOLS : (half + 1) * WCOLS]
        # permute columns: paired q/k blocks first, then v cols in head order
        w3 = w_c.reshape(D, NH_LOCAL, 3, HS)
        blocks = []
        for p in range(PAIRS):
            for qk in range(2):
                blocks.append(w3[:, 2 * p, qk, :])
                blocks.append(w3[:, 2 * p + 1, qk, :])
        for h in range(NH_LOCAL):
            blocks.append(w3[:, h, 2, :])
        w_c = np.concatenate(blocks, axis=1).astype(ml_dtypes.bfloat16)
        # per-partition qk bias: partitions 0-63 <- head 2p, 64-127 <- head 2p+1
        bqk = np.zeros((128, PAIRS, 2), dtype=np.float32)
        for p in range(PAIRS):
            for qk in range(2):
                bqk[0:64, p, qk] = b_c[(2 * p) * 192 + qk * 64 : (2 * p) * 192 + (qk + 1) * 64]
                bqk[64:128, p, qk] = b_c[(2 * p + 1) * 192 + qk * 64 : (2 * p + 1) * 192 + (qk + 1) * 64]
        bv = np.ascontiguousarray(
            b_c.reshape(NH_LOCAL, 3, HS)[:, 2, :].reshape(VCOLS)
        )
        in_maps.append(
            {
                "x": np.ascontiguousarray(x[b].T).astype(ml_dtypes.bfloat16),
                "w": w_c,
                "bqk": bqk,
                "bv": bv,
            }
        )
    return in_maps


def assemble_output(results):
    out = np.zeros((4, T, D), dtype=np.float32)
    for c in range(8):
        b, half = divmod(c, 2)
        out[b, :, half * VCOLS : (half + 1) * VCOLS] = results[c]["out"].T
    return out


def kernel(x, W_qkv, b_qkv):
    from concourse.bass_utils import run_bass_kernel_spmd

    nc = get_nc()
    in_maps = make_in_maps(x, W_qkv, b_qkv)
    res = run_bass_kernel_spmd(nc, in_maps, core_ids=list(range(8)))
    return assemble_output(res.results)


if __name__ == "__main__":
    xs = np.random.randn(4, T, D).astype(np.float32)
    Ws = (np.random.randn(D, 3 * D) / 32.0).astype(np.float32)
    bs = (np.random.randn(3 * D) * 0.02).astype(np.float32)
    o = kernel(xs, Ws, bs)
    print(o.shape, o.dtype)


# revision 17
# speedup vs baseline: 1.1370x; 1.0618x over previous
"""Fused QKV + multi-head attention kernel for Trainium2 (Bass/Tile), 8-core SPMD.

Problem: x[4, 2048, 1024] -> qkv = x @ W_qkv + b_qkv -> 16-head attention -> out[4, 2048, 1024].

Sharding (DP x TP): core c handles batch c//2 and head-group c%2 (8 of 16 heads),
so each core runs the qkv projection for its batch restricted to its heads'
columns of W_qkv, plus full attention for its 8 heads. No cross-core comm.

Per-core kernel design (bf16 matmuls, fp32 accumulation):
 - the host pre-transposes x during sharding, so the device loads xT[k, tok]
   directly with a casting DMA (f32->bf16); token-block-major DMA order lets
   the first k-projection chains start before the full x is resident.
 - qk projection computes qkv^T directly: lhsT = W tile, rhs = xT. Heads are
   processed in pairs: head 2p on SBUF partitions 0-63, head 2p+1 on 64-127,
   making the K=64 score matmuls row-tile onto independent 64-row PE tiles.
 - v projection computes v in [tok, feat] orientation with a ones-column per
   head so the attention A@V matmul also produces the softmax denominator.
 - softmax exp is the scalar-engine bottleneck (1 elem/lane/cycle, +352cyc
   per instruction, PSUM-read free dim capped at 1024 by the 2-bank tiles).
   To balance engines, each query-group's 16 key-tiles split two ways:
     * jj 9-15 "direct": ACT exp reads scores straight from PSUM [128,1024].
     * jj 0-8 "copied": DVE copies scores PSUM->SBUF bf16; the exp for those
       9 blocks runs one window later as two long-free ACT instructions
       ([128,5120] + [128,4096]) amortizing the per-instruction overhead.
   The A@V accumulation for a group therefore wraps across windows: start=
   jj9 (direct, this window), stop=jj8 (copied, next window). po double-
   buffering hands the two PSUM banks over mid-window.
 - output normalization (1/den broadcast multiply + v-bias) runs on GpSimd
   to keep DVE free for the score copies; the denominator reciprocal is
   computed 128-lane-wide via a DRAM bounce and stride-0 broadcast DMA.
"""

import sys

sys.path.insert(0, "/opt/trn_rl_repo")

import numpy as np
import ml_dtypes

T = 2048
D = 1024
NH_LOCAL = 8  # heads per core
HS = 64
WCOLS = NH_LOCAL * 3 * HS  # 1536
VCOLS = NH_LOCAL * HS  # 512
KT = D // 128  # 8 contraction tiles
PAIRS = NH_LOCAL // 2  # 4
IG = T // 512  # 4 query groups
JT = T // 128  # 16 key tiles
NCOPY = 11  # key tiles 0..10 take the copied (big-exp) path
NBIGA = 6  # copied blocks per first big exp ([128, 6144])

_CACHE: dict = {}


def _emit(tc, x_d, w_d, bqk_d, bv_d, out_d):
    import concourse.bass as bass
    from concourse import mybir
    from contextlib import ExitStack

    nc = tc.nc
    f32 = mybir.dt.float32
    bf16 = mybir.dt.bfloat16
    Exp = mybir.ActivationFunctionType.Exp

    ctx = ExitStack()
    pers = ctx.enter_context(tc.tile_pool(name="pers", bufs=1))
    qk_pool = ctx.enter_context(tc.tile_pool(name="qk", bufs=3))
    stage_pool = ctx.enter_context(tc.tile_pool(name="stage", bufs=2))
    bige_pool = ctx.enter_context(tc.tile_pool(name="bige", bufs=2))
    e_pool = ctx.enter_context(tc.tile_pool(name="epool", bufs=3))
    o_pool = ctx.enter_context(tc.tile_pool(name="opool", bufs=3))
    sm_pool = ctx.enter_context(tc.tile_pool(name="smpool", bufs=4))
    ot_pool = ctx.enter_context(tc.tile_pool(name="otpool", bufs=2))
    dr_pool = ctx.enter_context(tc.tile_pool(name="drpool", bufs=4, space="DRAM"))
    # PSUM: 8 banks = ps 2 (proj) + pss 2x2 (score staging) + po 2 (AV accum).
    ps_pool = ctx.enter_context(tc.tile_pool(name="pspool", bufs=2, space="PSUM"))
    pss_pool = ctx.enter_context(tc.tile_pool(name="psspool", bufs=2, space="PSUM"))
    po_pool = ctx.enter_context(tc.tile_pool(name="popool", bufs=2, space="PSUM"))

    # ---- DMA order: biases, pair-0 qk weights, then x token-block-major
    #      (first k chains start after ~1.5MB), v weights mid-stream, and the
    #      pairs-1-3 qk weights last (not needed until window 2).
    xt_sb = pers.tile([128, KT, T], bf16)  # xT[k, tok] per k-tile
    w_sb = pers.tile([128, KT, WCOLS], bf16)
    bqk_sb = pers.tile([128, PAIRS, 2], f32)  # [part, pair, q/k] per-partition bias
    nc.sync.dma_start(bqk_sb, bqk_d)
    bv_pp = pers.tile([64, NH_LOCAL], f32)  # v-bias, per-partition layout [d, head]
    nc.sync.dma_start(bv_pp, bv_d.rearrange("(h d) -> d h", d=HS))
    # v-bias pre-broadcast along queries so the output bias-add can run as a
    # gpsimd tensor_tensor (its tensor_scalar path is ~7.5us per call)
    bias_bc = pers.tile([64, NH_LOCAL, 512], f32)
    nc.vector.memset(bias_bc, 0.0)
    for h in range(NH_LOCAL):
        nc.vector.tensor_scalar_add(
            bias_bc[:, h, :], bias_bc[:, h, :], bv_pp[:, h : h + 1]
        )
    for kk in range(KT):
        nc.sync.dma_start(w_sb[:, kk, 0:256], w_d[kk * 128 : (kk + 1) * 128, 0:256])
    # x split across two otherwise-idle queues; v/rest-of-W on a third
    for g in range(IG):
        for kk in range(KT):
            q = nc.sync if kk % 2 == 0 else nc.scalar
            q.dma_start(
                xt_sb[:, kk, g * 512 : (g + 1) * 512],
                x_d[kk * 128 : (kk + 1) * 128, g * 512 : (g + 1) * 512],
            )
    for kk in range(KT):
        nc.gpsimd.dma_start(
            w_sb[:, kk, 1024:1536], w_d[kk * 128 : (kk + 1) * 128, 1024:1536]
        )
    for kk in range(KT):
        nc.gpsimd.dma_start(
            w_sb[:, kk, 256:1024], w_d[kk * 128 : (kk + 1) * 128, 256:1024]
        )

    # ---- v ones-columns ----
    v_sb = pers.tile([128, JT, NH_LOCAL, HS + 1], bf16)
    nc.vector.memset(v_sb[:, :, :, HS : HS + 1], 1.0)

    # w is host-permuted: cols [(p*2+qk)*128 : +128] = paired q/k block for pair p,
    # cols [1024:1536] = v columns in head order.
    def qk_chain(p, g, qk):
        ps = ps_pool.tile([128, 512], f32, tag="ps", name="psqk")
        c0 = (p * 2 + qk) * 128
        for kk in range(KT):
            nc.tensor.matmul(
                ps,
                w_sb[:, kk, c0 : c0 + 128],
                xt_sb[:, kk, g * 512 : (g + 1) * 512],
                start=(kk == 0),
                stop=(kk == KT - 1),
            )
        nc.vector.tensor_scalar_add(
            qk_tiles[p][:, qk, g * 512 : (g + 1) * 512], ps, bqk_sb[:, p, qk : qk + 1]
        )

    def v_chain(tt):
        ps = ps_pool.tile([128, 512], f32, tag="ps", name="psv")
        for kk in range(KT):
            nc.tensor.matmul(
                ps,
                xt_sb[:, kk, tt * 128 : (tt + 1) * 128],
                w_sb[:, kk, 1024:1536],
                start=(kk == 0),
                stop=(kk == KT - 1),
            )
        nc.vector.tensor_copy(
            v_sb[:, tt, :, 0:HS], ps.rearrange("p (h c) -> p h c", c=HS)
        )

    qk_tiles = [
        qk_pool.tile([128, 2, T], bf16, tag="qkt", name=f"qkt{_p}") for _p in range(PAIRS)
    ]

    def scores(p, ig, jj):
        """scores^T for both heads of pair p, key tile jj, query group ig."""
        qk_t = qk_tiles[p]
        ps = pss_pool.tile([128, 1024], f32, tag="pss")
        for h in range(2):
            base = 64 * h
            nc.tensor.matmul(
                ps[:, h * 512 : (h + 1) * 512],
                qk_t[base : base + 64, 1, jj * 128 : (jj + 1) * 128],
                qk_t[base : base + 64, 0, ig * 512 : (ig + 1) * 512],
                start=True,
                stop=True,
            )
        return ps

    def av(p, po, e_ap, jj, start, stop):
        """po[h] += [v|1]^T @ E for both heads; e_ap is [128, 1024] (h0|h1)."""
        for h in range(2):
            nc.tensor.matmul(
                po[h],
                v_sb[:, jj, 2 * p + h, :],
                e_ap[:, h * 512 : (h + 1) * 512],
                start=start,
                stop=stop,
            )

    def normalize(p, ig, po):
        """den-normalize po, add v-bias, store out^T (gpsimd does the muls)."""
        o_t = [o_pool.tile([65, 512], f32, tag="o", name=f"ot{_h}") for _h in range(2)]
        rcd_t = dr_pool.tile([2, 512], f32, tag="rcd")
        rct = sm_pool.tile([128, 8], f32, tag="rct")
        for h in range(2):
            nc.vector.tensor_copy(o_t[h], po[h])
            nc.sync.dma_start(rct[:, h * 4 : (h + 1) * 4], o_t[h][64:65, :])
        rcp = sm_pool.tile([128, 8], f32, tag="rcp")
        nc.vector.reciprocal(rcp, rct)
        for h in range(2):
            nc.sync.dma_start(rcd_t[h], rcp[:, h * 4 : (h + 1) * 4])
        for h in range(2):
            den_bc = sm_pool.tile([64, 512], f32, tag="denbc", name=f"dbc{h}")
            rcd_h = rcd_t[h]
            bc_src = bass.AP(
                tensor=rcd_h.tensor,
                offset=rcd_h.offset,
                ap=[[0, 64]] + list(rcd_h.ap),
            )
            nc.gpsimd.dma_start(den_bc, bc_src)
            ot = ot_pool.tile([64, 512], f32, tag="ot")
            nc.gpsimd.tensor_mul(ot, o_t[h][0:64, :], den_bc)
            nc.gpsimd.tensor_add(ot, ot, bias_bc[:, 2 * p + h, :])
            nc.sync.dma_start(
                out_d[(2 * p + h) * HS : (2 * p + h + 1) * HS, ig * 512 : (ig + 1) * 512],
                ot,
            )

    # ---------------- window pipeline ----------------
    # window w = (p, ig); carries over: copied-score stage + po of window w-1.
    windows = [(p, ig) for p in range(PAIRS) for ig in range(IG)]

    # PE filler chains per window (projection work woven into PE slack).
    # pair 0: k(g0..3)+q(g0) run in the prologue; v tiles 12-15 must precede
    # this window's direct AVs, v 0-11 the copied AVs that run early in w1.
    fillers = {w: [] for w in range(len(windows) + 1)}

    def F(w, fn, *a):
        import functools

        fillers[w].append(functools.partial(fn, *a))

    # window 0 runs all-direct (no previous group to overlap), so every v
    # tile must be ready in jj order for its AV; the fillers pop in order.
    for tt in range(JT):
        F(0, v_chain, tt)
    F(0, qk_chain, 0, 1, 0)
    F(1, qk_chain, 0, 2, 0)
    F(2, qk_chain, 0, 3, 0)
    for p in range(1, PAIRS):
        base = 4 * (p - 1)
        # next pair's k chains late in the previous pair, its q(g0) last
        F(base + 2, qk_chain, p, 0, 1)
        F(base + 2, qk_chain, p, 1, 1)
        F(base + 3, qk_chain, p, 2, 1)
        F(base + 3, qk_chain, p, 3, 1)
        F(base + 3, qk_chain, p, 0, 0)
        # own later q chains woven into own windows
        F(4 * p + 0, qk_chain, p, 1, 0)
        F(4 * p + 1, qk_chain, p, 2, 0)
        F(4 * p + 2, qk_chain, p, 3, 0)

    # prologue: pair-0 k chains for all query groups + q(g0)
    for g in range(IG):
        qk_chain(0, g, 1)
    qk_chain(0, 0, 0)

    prev = None  # (p, ig, po, stage)
    for w, (p, ig) in enumerate(windows):
        fill = list(fillers[w])

        def pop_fill(n=1):
            for _ in range(n):
                if fill:
                    fill.pop(0)()

        po = [
            po_pool.tile([65, 512], f32, tag="po", name=f"po{_h}") for _h in range(2)
        ]

        if w == 0:
            # all-direct first window: ACT has no carried-over big exps yet,
            # and the v chains must weave in jj order ahead of each AV.
            for jj in range(JT):
                ps = scores(p, ig, jj)
                e_t = e_pool.tile([128, 1024], bf16, tag="e")
                nc.scalar.activation(e_t, ps, Exp, scale=0.125)
                pop_fill()
                av(p, po, e_t, jj, start=(jj == 0), stop=(jj == JT - 1))
            while fill:
                fill.pop(0)()
            normalize(p, ig, po)
            prev = None
            continue

        stage = stage_pool.tile([128, NCOPY * 1024], bf16, tag="stage")

        # -- copied-path scores + DVE copies for this window, interleaved with
        #    the big exps + AVs that finish the PREVIOUS window's group. The
        #    AVc bursts are spread between score matmuls so the PE's FIFO
        #    never blocks score production for long.
        bige_a = bige_b = None
        if prev is not None:
            pp, pig, ppo, pstage = prev
            bige_a = bige_pool.tile([128, NBIGA * 1024], bf16, tag="bige")
            nc.scalar.activation(bige_a, pstage[:, : NBIGA * 1024], Exp, scale=0.125)
        for jj in range(NCOPY):
            ps = scores(p, ig, jj)
            nc.vector.tensor_copy(stage[:, jj * 1024 : (jj + 1) * 1024], ps)
            pop_fill()
            if prev is not None:
                if jj == 5:
                    bige_b = bige_pool.tile(
                        [128, (NCOPY - NBIGA) * 1024], bf16, tag="bige"
                    )
                    nc.scalar.activation(
                        bige_b, pstage[:, NBIGA * 1024 :], Exp, scale=0.125
                    )
                if 6 <= jj < 9:
                    bs = [2 * (jj - 6), 2 * (jj - 6) + 1]
                elif jj == 9:
                    bs = [6, 7]
                elif jj == 10:
                    bs = [8, 9, 10]
                else:
                    bs = []
                for b in bs:
                    if b < NBIGA:
                        e_ap = bige_a[:, b * 1024 : (b + 1) * 1024]
                    else:
                        e_ap = bige_b[:, (b - NBIGA) * 1024 : (b - NBIGA + 1) * 1024]
                    av(pp, ppo, e_ap, b, start=False, stop=(b == NCOPY - 1))
        if prev is not None:
            normalize(pp, pig, ppo)

        # -- direct path: jj 11..15, exp straight from PSUM, AV immediately
        for jj in range(NCOPY, JT):
            ps = scores(p, ig, jj)
            e_t = e_pool.tile([128, 1024], bf16, tag="e")
            nc.scalar.activation(e_t, ps, Exp, scale=0.125)
            av(p, po, e_t, jj, start=(jj == NCOPY), stop=False)
            pop_fill()
        while fill:
            fill.pop(0)()

        prev = (p, ig, po, stage)

    # ---- drain the last window's copied path ----
    pp, pig, ppo, pstage = prev
    bige_a = bige_pool.tile([128, NBIGA * 1024], bf16, tag="bige")
    nc.scalar.activation(bige_a, pstage[:, : NBIGA * 1024], Exp, scale=0.125)
    for b in range(NBIGA):
        av(pp, ppo, bige_a[:, b * 1024 : (b + 1) * 1024], b, start=False, stop=False)
    bige_b = bige_pool.tile([128, (NCOPY - NBIGA) * 1024], bf16, tag="bige")
    nc.scalar.activation(bige_b, pstage[:, NBIGA * 1024 :], Exp, scale=0.125)
    for b in range(NBIGA, NCOPY):
        av(pp, ppo, bige_b[:, (b - NBIGA) * 1024 : (b - NBIGA + 1) * 1024], b,
           start=False, stop=(b == NCOPY - 1))
    normalize(pp, pig, ppo)
    ctx.close()


def _build():
    import concourse.tile as tile
    from concourse import bacc, mybir

    f32 = mybir.dt.float32
    nc = bacc.Bacc("TRN2", target_bir_lowering=False, debug=False, num_devices=8)
    x_d = nc.dram_tensor("x", [D, T], mybir.dt.bfloat16, kind="ExternalInput").ap()
    w_d = nc.dram_tensor("w", [D, WCOLS], mybir.dt.bfloat16, kind="ExternalInput").ap()
    bqk_d = nc.dram_tensor("bqk", [128, PAIRS, 2], f32, kind="ExternalInput").ap()
    bv_d = nc.dram_tensor("bv", [VCOLS], f32, kind="ExternalInput").ap()
    out_d = nc.dram_tensor("out", [VCOLS, T], f32, kind="ExternalOutput").ap()
    with tile.TileContext(nc) as tc:
        _emit(tc, x_d, w_d, bqk_d, bv_d, out_d)
    nc.compile()
    return nc


def get_nc():
    if "nc" not in _CACHE:
        _CACHE["nc"] = _build()
    return _CACHE["nc"]


def make_in_maps(x, W_qkv, b_qkv):
    """Shard full inputs into 8 per-core input maps."""
    x = np.asarray(x, dtype=np.float32)
    W_qkv = np.asarray(W_qkv, dtype=np.float32)
    b_qkv = np.asarray(b_qkv, dtype=np.float32)
    in_maps = []
    for c in range(8):
        b, half = divmod(c, 2)
        w_c = W_qkv[:, half * WCOLS : (half + 1) * WCOLS]
        b_c = b_qkv[half * WCOLS : (half + 1) * WCOLS]
        # permute columns: paired q/k blocks first, then v cols in head order
        w3 = w_c.reshape(D, NH_LOCAL, 3, HS)
        blocks = []
        for p in range(PAIRS):
            for qk in range(2):
                blocks.append(w3[:, 2 * p, qk, :])
                blocks.append(w3[:, 2 * p + 1, qk, :])
        for h in range(NH_LOCAL):
            blocks.append(w3[:, h, 2, :])
        w_c = np.concatenate(blocks, axis=1).astype(ml_dtypes.bfloat16)
        # per-partition qk bias: partitions 0-63 <- head 2p, 64-127 <- head 2p+1
        bqk = np.zeros((128, PAIRS, 2), dtype=np.float32)
        for p in range(PAIRS):
            for qk in range(2):
                bqk[0:64, p, qk] = b_c[(2 * p) * 192 + qk * 64 : (2 * p) * 192 + (qk + 1) * 64]
                bqk[64:128, p, qk] = b_c[(2 * p + 1) * 192 + qk * 64 : (2 * p + 1) * 192 + (qk + 1) * 64]
        bv = np.ascontiguousarray(
            b_c.reshape(NH_LOCAL, 3, HS)[:, 2, :].reshape(VCOLS)
        )
        in_maps.append(
            {
                "x": np.ascontiguousarray(x[b].T).astype(ml_dtypes.bfloat16),
                "w": w_c,
                "bqk": bqk,
                "bv": bv,
            }
        )
    return in_maps


def assemble_output(results):
    out = np.zeros((4, T, D), dtype=np.float32)
    for c in range(8):
        b, half = divmod(c, 2)
        out[b, :, half * VCOLS : (half + 1) * VCOLS] = results[c]["out"].T
    return out


def kernel(x, W_qkv, b_qkv):
    from concourse.bass_utils import run_bass_kernel_spmd

    nc = get_nc()
    in_maps = make_in_maps(x, W_qkv, b_qkv)
    res = run_bass_kernel_spmd(nc, in_maps, core_ids=list(range(8)))
    return assemble_output(res.results)


if __name__ == "__main__":
    xs = np.random.randn(4, T, D).astype(np.float32)
    Ws = (np.random.randn(D, 3 * D) / 32.0).astype(np.float32)
    bs = (np.random.randn(3 * D) * 0.02).astype(np.float32)
    o = kernel(xs, Ws, bs)
    print(o.shape, o.dtype)


# revision 19
# speedup vs baseline: 1.1677x; 1.0270x over previous
"""Fused QKV + multi-head attention kernel for Trainium2 (Bass/Tile), 8-core SPMD.

Problem: x[4, 2048, 1024] -> qkv = x @ W_qkv + b_qkv -> 16-head attention -> out[4, 2048, 1024].

Sharding (DP x TP): core c handles batch c//2 and head-group c%2 (8 of 16 heads),
so each core runs the qkv projection for its batch restricted to its heads'
columns of W_qkv, plus full attention for its 8 heads. No cross-core comm.

Per-core kernel design (bf16 matmuls, fp32 accumulation):
 - the host pre-transposes x during sharding, so the device loads xT[k, tok]
   directly with a casting DMA (f32->bf16); token-block-major DMA order lets
   the first k-projection chains start before the full x is resident.
 - qk projection computes qkv^T directly: lhsT = W tile, rhs = xT. Heads are
   processed in pairs: head 2p on SBUF partitions 0-63, head 2p+1 on 64-127,
   making the K=64 score matmuls row-tile onto independent 64-row PE tiles.
 - v projection computes v in [tok, feat] orientation with a ones-column per
   head so the attention A@V matmul also produces the softmax denominator.
 - softmax exp is the scalar-engine bottleneck (1 elem/lane/cycle, +352cyc
   per instruction, PSUM-read free dim capped at 1024 by the 2-bank tiles).
   To balance engines, each query-group's 16 key-tiles split two ways:
     * jj 9-15 "direct": ACT exp reads scores straight from PSUM [128,1024].
     * jj 0-8 "copied": DVE copies scores PSUM->SBUF bf16; the exp for those
       9 blocks runs one window later as two long-free ACT instructions
       ([128,5120] + [128,4096]) amortizing the per-instruction overhead.
   The A@V accumulation for a group therefore wraps across windows: start=
   jj9 (direct, this window), stop=jj8 (copied, next window). po double-
   buffering hands the two PSUM banks over mid-window.
 - output normalization (1/den broadcast multiply + v-bias) runs on GpSimd
   to keep DVE free for the score copies; the denominator reciprocal is
   computed 128-lane-wide via a DRAM bounce and stride-0 broadcast DMA.
"""

import sys

sys.path.insert(0, "/opt/trn_rl_repo")

import numpy as np
import ml_dtypes

T = 2048
D = 1024
NH_LOCAL = 8  # heads per core
HS = 64
WCOLS = NH_LOCAL * 3 * HS  # 1536
VCOLS = NH_LOCAL * HS  # 512
KT = D // 128  # 8 contraction tiles
PAIRS = NH_LOCAL // 2  # 4
IG = T // 512  # 4 query groups
JT = T // 128  # 16 key tiles
NCOPY = 11  # key tiles 0..10 take the copied (big-exp) path
NBIGA = 6  # copied blocks per first big exp ([128, 6144])

_CACHE: dict = {}


def _emit(tc, x_d, w_d, bqk_d, bv_d, out_d):
    import concourse.bass as bass
    from concourse import mybir
    from contextlib import ExitStack

    nc = tc.nc
    f32 = mybir.dt.float32
    bf16 = mybir.dt.bfloat16
    Exp = mybir.ActivationFunctionType.Exp

    ctx = ExitStack()
    pers = ctx.enter_context(tc.tile_pool(name="pers", bufs=1))
    qk_pool = ctx.enter_context(tc.tile_pool(name="qk", bufs=3))
    stage_pool = ctx.enter_context(tc.tile_pool(name="stage", bufs=2))
    bige_pool = ctx.enter_context(tc.tile_pool(name="bige", bufs=2))
    e_pool = ctx.enter_context(tc.tile_pool(name="epool", bufs=3))
    o_pool = ctx.enter_context(tc.tile_pool(name="opool", bufs=3))
    sm_pool = ctx.enter_context(tc.tile_pool(name="smpool", bufs=4))
    ot_pool = ctx.enter_context(tc.tile_pool(name="otpool", bufs=2))
    dr_pool = ctx.enter_context(tc.tile_pool(name="drpool", bufs=4, space="DRAM"))
    # PSUM: 8 banks = ps 2 (proj) + pss 2x2 (score staging) + po 2 (AV accum).
    ps_pool = ctx.enter_context(tc.tile_pool(name="pspool", bufs=2, space="PSUM"))
    pss_pool = ctx.enter_context(tc.tile_pool(name="psspool", bufs=2, space="PSUM"))
    po_pool = ctx.enter_context(tc.tile_pool(name="popool", bufs=2, space="PSUM"))

    # ---- DMA order: biases, pair-0 qk weights, then x token-block-major
    #      (first k chains start after ~1.5MB), v weights mid-stream, and the
    #      pairs-1-3 qk weights last (not needed until window 2).
    xt_sb = pers.tile([128, KT, T], bf16)  # xT[k, tok] per k-tile
    w_sb = pers.tile([128, KT, WCOLS], bf16)
    bqk_sb = pers.tile([128, PAIRS, 2], f32)  # [part, pair, q/k] per-partition bias
    nc.sync.dma_start(bqk_sb, bqk_d)
    bv_pp = pers.tile([64, NH_LOCAL], f32)  # v-bias, per-partition layout [d, head]
    nc.sync.dma_start(bv_pp, bv_d.rearrange("(h d) -> d h", d=HS))
    # v-bias pre-broadcast along queries so the output bias-add can run as a
    # gpsimd tensor_tensor (its tensor_scalar path is ~7.5us per call)
    bias_bc = pers.tile([64, NH_LOCAL, 512], f32)
    nc.vector.memset(bias_bc, 0.0)
    for h in range(NH_LOCAL):
        nc.vector.tensor_scalar_add(
            bias_bc[:, h, :], bias_bc[:, h, :], bv_pp[:, h : h + 1]
        )
    for kk in range(KT):
        nc.sync.dma_start(w_sb[:, kk, 0:256], w_d[kk * 128 : (kk + 1) * 128, 0:256])
    # x split across two otherwise-idle queues; v/rest-of-W on a third
    for g in range(IG):
        for kk in range(KT):
            q = nc.sync if kk % 2 == 0 else nc.scalar
            q.dma_start(
                xt_sb[:, kk, g * 512 : (g + 1) * 512],
                x_d[kk * 128 : (kk + 1) * 128, g * 512 : (g + 1) * 512],
            )
    for kk in range(KT):
        nc.gpsimd.dma_start(
            w_sb[:, kk, 1024:1536], w_d[kk * 128 : (kk + 1) * 128, 1024:1536]
        )
    # pairs 1-3 qk weights ride the scalar queue AFTER the x blocks so they
    # don't compete for HBM bandwidth during the latency-critical x load
    # (first needed by window 2, ~50us in).
    for kk in range(KT):
        nc.scalar.dma_start(
            w_sb[:, kk, 256:1024], w_d[kk * 128 : (kk + 1) * 128, 256:1024]
        )

    # ---- v ones-columns ----
    v_sb = pers.tile([128, JT, NH_LOCAL, HS + 1], bf16)
    nc.vector.memset(v_sb[:, :, :, HS : HS + 1], 1.0)

    # w is host-permuted: cols [(p*2+qk)*128 : +128] = paired q/k block for pair p,
    # cols [1024:1536] = v columns in head order.
    def qk_chain(p, g, qk):
        ps = ps_pool.tile([128, 512], f32, tag="ps", name="psqk")
        c0 = (p * 2 + qk) * 128
        for kk in range(KT):
            nc.tensor.matmul(
                ps,
                w_sb[:, kk, c0 : c0 + 128],
                xt_sb[:, kk, g * 512 : (g + 1) * 512],
                start=(kk == 0),
                stop=(kk == KT - 1),
            )
        nc.vector.tensor_scalar_add(
            qk_tiles[p][:, qk, g * 512 : (g + 1) * 512], ps, bqk_sb[:, p, qk : qk + 1]
        )

    def v_chain(tt):
        ps = ps_pool.tile([128, 512], f32, tag="ps", name="psv")
        for kk in range(KT):
            nc.tensor.matmul(
                ps,
                xt_sb[:, kk, tt * 128 : (tt + 1) * 128],
                w_sb[:, kk, 1024:1536],
                start=(kk == 0),
                stop=(kk == KT - 1),
            )
        nc.vector.tensor_copy(
            v_sb[:, tt, :, 0:HS], ps.rearrange("p (h c) -> p h c", c=HS)
        )

    qk_tiles = [
        qk_pool.tile([128, 2, T], bf16, tag="qkt", name=f"qkt{_p}") for _p in range(PAIRS)
    ]

    def scores(p, ig, jj):
        """scores^T for both heads of pair p, key tile jj, query group ig."""
        qk_t = qk_tiles[p]
        ps = pss_pool.tile([128, 1024], f32, tag="pss")
        for h in range(2):
            base = 64 * h
            nc.tensor.matmul(
                ps[:, h * 512 : (h + 1) * 512],
                qk_t[base : base + 64, 1, jj * 128 : (jj + 1) * 128],
                qk_t[base : base + 64, 0, ig * 512 : (ig + 1) * 512],
                start=True,
                stop=True,
            )
        return ps

    def av(p, po, e_ap, jj, start, stop):
        """po[h] += [v|1]^T @ E for both heads; e_ap is [128, 1024] (h0|h1)."""
        for h in range(2):
            nc.tensor.matmul(
                po[h],
                v_sb[:, jj, 2 * p + h, :],
                e_ap[:, h * 512 : (h + 1) * 512],
                start=start,
                stop=stop,
            )

    def normalize(p, ig, po):
        """den-normalize po, add v-bias, store out^T (gpsimd does the muls)."""
        o_t = [o_pool.tile([65, 512], f32, tag="o", name=f"ot{_h}") for _h in range(2)]
        rcd_t = dr_pool.tile([2, 512], f32, tag="rcd")
        rct = sm_pool.tile([128, 8], f32, tag="rct")
        for h in range(2):
            nc.vector.tensor_copy(o_t[h], po[h])
            nc.sync.dma_start(rct[:, h * 4 : (h + 1) * 4], o_t[h][64:65, :])
        rcp = sm_pool.tile([128, 8], f32, tag="rcp")
        nc.vector.reciprocal(rcp, rct)
        for h in range(2):
            nc.sync.dma_start(rcd_t[h], rcp[:, h * 4 : (h + 1) * 4])
        for h in range(2):
            den_bc = sm_pool.tile([64, 512], f32, tag="denbc", name=f"dbc{h}")
            rcd_h = rcd_t[h]
            bc_src = bass.AP(
                tensor=rcd_h.tensor,
                offset=rcd_h.offset,
                ap=[[0, 64]] + list(rcd_h.ap),
            )
            nc.gpsimd.dma_start(den_bc, bc_src)
            ot = ot_pool.tile([64, 512], f32, tag="ot")
            nc.gpsimd.tensor_mul(ot, o_t[h][0:64, :], den_bc)
            nc.gpsimd.tensor_add(ot, ot, bias_bc[:, 2 * p + h, :])
            nc.sync.dma_start(
                out_d[(2 * p + h) * HS : (2 * p + h + 1) * HS, ig * 512 : (ig + 1) * 512],
                ot,
            )

    # ---------------- window pipeline ----------------
    # window w = (p, ig); carries over: copied-score stage + po of window w-1.
    windows = [(p, ig) for p in range(PAIRS) for ig in range(IG)]

    # PE filler chains per window (projection work woven into PE slack).
    # pair 0: k(g0..3)+q(g0) run in the prologue; v tiles 12-15 must precede
    # this window's direct AVs, v 0-11 the copied AVs that run early in w1.
    fillers = {w: [] for w in range(len(windows) + 1)}

    def F(w, fn, *a):
        import functools

        fillers[w].append(functools.partial(fn, *a))

    # window 0 runs all-direct (no previous group to overlap), so every v
    # tile must be ready in jj order for its AV; the fillers pop in order.
    for tt in range(JT):
        F(0, v_chain, tt)
    F(0, qk_chain, 0, 1, 0)
    F(1, qk_chain, 0, 2, 0)
    F(2, qk_chain, 0, 3, 0)
    for p in range(1, PAIRS):
        base = 4 * (p - 1)
        # next pair's k chains late in the previous pair, its q(g0) last
        F(base + 2, qk_chain, p, 0, 1)
        F(base + 2, qk_chain, p, 1, 1)
        F(base + 3, qk_chain, p, 2, 1)
        F(base + 3, qk_chain, p, 3, 1)
        F(base + 3, qk_chain, p, 0, 0)
        # own later q chains woven into own windows
        F(4 * p + 0, qk_chain, p, 1, 0)
        F(4 * p + 1, qk_chain, p, 2, 0)
        F(4 * p + 2, qk_chain, p, 3, 0)

    # prologue: pair-0 k chains for all query groups + q(g0)
    for g in range(IG):
        qk_chain(0, g, 1)
    qk_chain(0, 0, 0)

    prev = None  # (p, ig, po, stage)
    for w, (p, ig) in enumerate(windows):
        fill = list(fillers[w])

        def pop_fill(n=1):
            for _ in range(n):
                if fill:
                    fill.pop(0)()

        po = [
            po_pool.tile([65, 512], f32, tag="po", name=f"po{_h}") for _h in range(2)
        ]

        if w == 0:
            # all-direct first window: ACT has no carried-over big exps yet,
            # and the v chains must weave in jj order ahead of each AV.
            for jj in range(JT):
                ps = scores(p, ig, jj)
                e_t = e_pool.tile([128, 1024], bf16, tag="e")
                nc.scalar.activation(e_t, ps, Exp, scale=0.125)
                pop_fill()
                av(p, po, e_t, jj, start=(jj == 0), stop=(jj == JT - 1))
            while fill:
                fill.pop(0)()
            normalize(p, ig, po)
            prev = None
            continue

        stage = stage_pool.tile([128, NCOPY * 1024], bf16, tag="stage")

        # -- copied-path scores + DVE copies for this window, interleaved with
        #    the big exps + AVs that finish the PREVIOUS window's group. The
        #    AVc bursts are spread between score matmuls so the PE's FIFO
        #    never blocks score production for long.
        bige_a = bige_b = None
        if prev is not None:
            pp, pig, ppo, pstage = prev
            bige_a = bige_pool.tile([128, NBIGA * 1024], bf16, tag="bige")
            nc.scalar.activation(bige_a, pstage[:, : NBIGA * 1024], Exp, scale=0.125)
        for jj in range(NCOPY):
            ps = scores(p, ig, jj)
            nc.vector.tensor_copy(stage[:, jj * 1024 : (jj + 1) * 1024], ps)
            # filler chains only mid-window: bunched at the start they delay
            # score production and starve the DVE copy train
            if jj in (4, 7, 10):
                pop_fill()
            if prev is not None:
                if jj == 5:
                    bige_b = bige_pool.tile(
                        [128, (NCOPY - NBIGA) * 1024], bf16, tag="bige"
                    )
                    nc.scalar.activation(
                        bige_b, pstage[:, NBIGA * 1024 :], Exp, scale=0.125
                    )
                if 6 <= jj < 9:
                    bs = [2 * (jj - 6), 2 * (jj - 6) + 1]
                elif jj == 9:
                    bs = [6, 7]
                elif jj == 10:
                    bs = [8, 9, 10]
                else:
                    bs = []
                for b in bs:
                    if b < NBIGA:
                        e_ap = bige_a[:, b * 1024 : (b + 1) * 1024]
                    else:
                        e_ap = bige_b[:, (b - NBIGA) * 1024 : (b - NBIGA + 1) * 1024]
                    av(pp, ppo, e_ap, b, start=False, stop=(b == NCOPY - 1))
        if prev is not None:
            normalize(pp, pig, ppo)

        # -- direct path: jj 11..15, exp straight from PSUM, AV immediately
        for jj in range(NCOPY, JT):
            ps = scores(p, ig, jj)
            e_t = e_pool.tile([128, 1024], bf16, tag="e")
            nc.scalar.activation(e_t, ps, Exp, scale=0.125)
            av(p, po, e_t, jj, start=(jj == NCOPY), stop=False)
            pop_fill()
        while fill:
            fill.pop(0)()

        prev = (p, ig, po, stage)

    # ---- drain the last window's copied path ----
    pp, pig, ppo, pstage = prev
    bige_a = bige_pool.tile([128, NBIGA * 1024], bf16, tag="bige")
    nc.scalar.activation(bige_a, pstage[:, : NBIGA * 1024], Exp, scale=0.125)
    for b in range(NBIGA):
        av(pp, ppo, bige_a[:, b * 1024 : (b + 1) * 1024], b, start=False, stop=False)
    bige_b = bige_pool.tile([128, (NCOPY - NBIGA) * 1024], bf16, tag="bige")
    nc.scalar.activation(bige_b, pstage[:, NBIGA * 1024 :], Exp, scale=0.125)
    for b in range(NBIGA, NCOPY):
        av(pp, ppo, bige_b[:, (b - NBIGA) * 1024 : (b - NBIGA + 1) * 1024], b,
           start=False, stop=(b == NCOPY - 1))
    normalize(pp, pig, ppo)
    ctx.close()


def _build():
    import concourse.tile as tile
    from concourse import bacc, mybir

    f32 = mybir.dt.float32
    nc = bacc.Bacc("TRN2", target_bir_lowering=False, debug=False, num_devices=8)
    x_d = nc.dram_tensor("x", [D, T], mybir.dt.bfloat16, kind="ExternalInput").ap()
    w_d = nc.dram_tensor("w", [D, WCOLS], mybir.dt.bfloat16, kind="ExternalInput").ap()
    bqk_d = nc.dram_tensor("bqk", [128, PAIRS, 2], f32, kind="ExternalInput").ap()
    bv_d = nc.dram_tensor("bv", [VCOLS], f32, kind="ExternalInput").ap()
    out_d = nc.dram_tensor("out", [VCOLS, T], f32, kind="ExternalOutput").ap()
    with tile.TileContext(nc) as tc:
        _emit(tc, x_d, w_d, bqk_d, bv_d, out_d)
    nc.compile()
    return nc


def get_nc():
    if "nc" not in _CACHE:
        _CACHE["nc"] = _build()
    return _CACHE["nc"]


def make_in_maps(x, W_qkv, b_qkv):
    """Shard full inputs into 8 per-core input maps."""
    x = np.asarray(x, dtype=np.float32)
    W_qkv = np.asarray(W_qkv, dtype=np.float32)
    b_qkv = np.asarray(b_qkv, dtype=np.float32)
    in_maps = []
    for c in range(8):
        b, half = divmod(c, 2)
        w_c = W_qkv[:, half * WCOLS : (half + 1) * WCOLS]
        b_c = b_qkv[half * WCOLS : (half + 1) * WCOLS]
        # permute columns: paired q/k blocks first, then v cols in head order
        w3 = w_c.reshape(D, NH_LOCAL, 3, HS)
        blocks = []
        for p in range(PAIRS):
            for qk in range(2):
                blocks.append(w3[:, 2 * p, qk, :])
                blocks.append(w3[:, 2 * p + 1, qk, :])
        for h in range(NH_LOCAL):
            blocks.append(w3[:, h, 2, :])
        w_c = np.concatenate(blocks, axis=1).astype(ml_dtypes.bfloat16)
        # per-partition qk bias: partitions 0-63 <- head 2p, 64-127 <- head 2p+1
        bqk = np.zeros((128, PAIRS, 2), dtype=np.float32)
        for p in range(PAIRS):
            for qk in range(2):
                bqk[0:64, p, qk] = b_c[(2 * p) * 192 + qk * 64 : (2 * p) * 192 + (qk + 1) * 64]
                bqk[64:128, p, qk] = b_c[(2 * p + 1) * 192 + qk * 64 : (2 * p + 1) * 192 + (qk + 1) * 64]
        bv = np.ascontiguousarray(
            b_c.reshape(NH_LOCAL, 3, HS)[:, 2, :].reshape(VCOLS)
        )
        in_maps.append(
            {
                "x": np.ascontiguousarray(x[b].T).astype(ml_dtypes.bfloat16),
                "w": w_c,
                "bqk": bqk,
                "bv": bv,
            }
        )
    return in_maps


def assemble_output(results):
    out = np.zeros((4, T, D), dtype=np.float32)
    for c in range(8):
        b, half = divmod(c, 2)
        out[b, :, half * VCOLS : (half + 1) * VCOLS] = results[c]["out"].T
    return out


def kernel(x, W_qkv, b_qkv):
    from concourse.bass_utils import run_bass_kernel_spmd

    nc = get_nc()
    in_maps = make_in_maps(x, W_qkv, b_qkv)
    res = run_bass_kernel_spmd(nc, in_maps, core_ids=list(range(8)))
    return assemble_output(res.results)


if __name__ == "__main__":
    xs = np.random.randn(4, T, D).astype(np.float32)
    Ws = (np.random.randn(D, 3 * D) / 32.0).astype(np.float32)
    bs = (np.random.randn(3 * D) * 0.02).astype(np.float32)
    o = kernel(xs, Ws, bs)
    print(o.shape, o.dtype)


# revision 23
# speedup vs baseline: 1.4428x; 1.2356x over previous
"""Fused QKV + multi-head attention kernel for Trainium2 (Bass/Tile), 8-core SPMD.

Problem: x[4, 2048, 1024] -> qkv = x @ W_qkv + b_qkv -> 16-head attention -> out[4, 2048, 1024].

Sharding (DP x TP): core c handles batch c//2 and head-group c%2 (8 of 16 heads),
so each core runs the qkv projection for its batch restricted to its heads'
columns of W_qkv, plus full attention for its 8 heads. No cross-core comm.

Per-core kernel design (bf16 matmuls, fp32 accumulation):
 - the host pre-transposes x during sharding, so the device loads xT[k, tok]
   directly with a casting DMA (f32->bf16); no on-device transposes anywhere.
 - qk projection computes qkv^T directly: lhsT = W tile (layout [k, feat] as
   stored), rhs = xT. Heads are processed in pairs: head 2p lives in SBUF
   partitions 0-63 and head 2p+1 in partitions 64-127 (both q^T and k^T), which
   makes the K=64 score matmuls legal (lhsT/rhs share a base partition) and
   lets the PE run the two heads on independent 64-row array tiles.
 - v projection computes v in [tok, feat] orientation (lhsT = xT stationary,
   rhs = W v-columns), with a ones-column appended per head so the attention
   A@V matmul also produces the softmax denominator.
 - scores^T = k^T.T @ q^T accumulate in PSUM; exp (with the 1/8 scale folded
   into the ACT affine) reads [128, 2048] PSUM spans to amortize ACT overhead;
   no max-subtraction (scores ~ N(0,1), exp never overflows fp32/bf16).
 - out^T[65, 512] = [v | 1].T @ E accumulates over key tiles; the denominator
   row is bounced through DRAM into a [128, 4] layout for a 128-lane reciprocal,
   broadcast back across partitions with a stride-0 DMA, and the normalized
   out^T (+ v-bias, added after normalization -- exact) is stored transposed;
   the host un-transposes during unsharding.
"""

import sys

sys.path.insert(0, "/opt/trn_rl_repo")

import numpy as np
import ml_dtypes

T = 2048
D = 1024
NH_LOCAL = 8  # heads per core
HS = 64
WCOLS = NH_LOCAL * 3 * HS  # 1536
VCOLS = NH_LOCAL * HS  # 512
KT = D // 128  # 8 contraction tiles
PAIRS = NH_LOCAL // 2  # 4
IG = T // 512  # 4 query groups
JT = T // 128  # 16 key tiles

_CACHE: dict = {}


def _emit(tc, x_d, w_d, bqk_d, bv_d, out_d):
    import concourse.bass as bass
    from concourse import mybir
    from contextlib import ExitStack

    nc = tc.nc
    f32 = mybir.dt.float32
    bf16 = mybir.dt.bfloat16
    Exp = mybir.ActivationFunctionType.Exp

    ctx = ExitStack()
    pers = ctx.enter_context(tc.tile_pool(name="pers", bufs=1))
    qk_pool = ctx.enter_context(tc.tile_pool(name="qk", bufs=PAIRS))
    e_pool = ctx.enter_context(tc.tile_pool(name="epool", bufs=3))
    o_pool = ctx.enter_context(tc.tile_pool(name="opool", bufs=3))
    sm_pool = ctx.enter_context(tc.tile_pool(name="smpool", bufs=4))
    ot_pool = ctx.enter_context(tc.tile_pool(name="otpool", bufs=4))
    dr_pool = ctx.enter_context(tc.tile_pool(name="drpool", bufs=4, space="DRAM"))
    # PSUM: 8 banks total = ps 2 (proj) + pss 2x2 (scores/exp staging) + po 2 (AV accum).
    # pss double-buffering is what keeps the scores->exp->AV pipeline from
    # serializing (a single buffer stalls the PE ~2.7us per exp group, which
    # also re-throttles the HAM clock gate to 1.2 GHz).
    ps_pool = ctx.enter_context(tc.tile_pool(name="pspool", bufs=2, space="PSUM"))
    pss_pool = ctx.enter_context(tc.tile_pool(name="psspool", bufs=2, space="PSUM"))
    po_pool = ctx.enter_context(tc.tile_pool(name="popool", bufs=2, space="PSUM"))

    # ---- DMA order: biases, pair-0 qk weights, v weights, then x token-
    #      block-major split over two queues (k chains start at ~1.5MB in);
    #      pairs 1-3 qk weights last, on the scalar queue behind x, so they
    #      don't compete for HBM during the latency-critical load.
    xt_sb = pers.tile([128, KT, T], bf16)  # xT[k, tok] per k-tile
    w_sb = pers.tile([128, KT, WCOLS], bf16)
    bqk_sb = pers.tile([128, PAIRS, 2], f32)  # [part, pair, q/k] per-partition bias
    nc.sync.dma_start(bqk_sb, bqk_d)
    bv_pp = pers.tile([64, NH_LOCAL], f32)  # v-bias, per-partition layout [d, head]
    nc.sync.dma_start(bv_pp, bv_d.rearrange("(h d) -> d h", d=HS))
    for kk in range(KT):
        nc.sync.dma_start(w_sb[:, kk, 0:256], w_d[kk * 128 : (kk + 1) * 128, 0:256])
    for kk in range(KT):
        nc.gpsimd.dma_start(
            w_sb[:, kk, 1024:WCOLS], w_d[kk * 128 : (kk + 1) * 128, 1024:WCOLS]
        )
    for g in range(IG):
        for kk in range(KT):
            q = nc.sync if kk % 2 == 0 else nc.scalar
            q.dma_start(
                xt_sb[:, kk, g * 512 : (g + 1) * 512],
                x_d[kk * 128 : (kk + 1) * 128, g * 512 : (g + 1) * 512],
            )
    for kk in range(KT):
        nc.scalar.dma_start(
            w_sb[:, kk, 256:1024], w_d[kk * 128 : (kk + 1) * 128, 256:1024]
        )

    # ---- v ones-columns ----
    v_sb = pers.tile([128, JT, NH_LOCAL, HS + 1], bf16)
    nc.vector.memset(v_sb[:, :, :, HS : HS + 1], 1.0)

    # w is host-permuted: cols [(p*2+qk)*128 : +128] = paired q/k block for pair p,
    # cols [1024:1536] = v columns in head order (walrus requires single-free-dim
    # matmul operand APs, so the pairing permutation happens host-side).
    def qk_chain(p, qk_t, g, qk):
        # one projection chain: q^T or k^T for pair p, token group g
        ps = ps_pool.tile([128, 512], f32, tag="ps", name="psqk")
        c0 = (p * 2 + qk) * 128
        for kk in range(KT):
            nc.tensor.matmul(
                ps,
                w_sb[:, kk, c0 : c0 + 128],
                xt_sb[:, kk, g * 512 : (g + 1) * 512],
                start=(kk == 0),
                stop=(kk == KT - 1),
            )
        nc.vector.tensor_scalar_add(
            qk_t[:, qk, g * 512 : (g + 1) * 512], ps, bqk_sb[:, p, qk : qk + 1]
        )

    def emit_qk_proj(p, qk_t):
        # psum partitions 0-63 <- head 2p, 64-127 <- head 2p+1
        for g in range(IG):
            for qk in range(2):
                qk_chain(p, qk_t, g, qk)

    def emit_v_chain(tt):
        # one v-projection chain: v[tok tile tt, all heads] = xT.T @ Wv
        ps = ps_pool.tile([128, 512], f32, tag="ps", name="psv")
        for kk in range(KT):
            nc.tensor.matmul(
                ps,
                xt_sb[:, kk, tt * 128 : (tt + 1) * 128],
                w_sb[:, kk, 1024:1536],
                start=(kk == 0),
                stop=(kk == KT - 1),
            )
        nc.vector.tensor_copy(
            v_sb[:, tt, :, 0:HS], ps.rearrange("p (h c) -> p h c", c=HS)
        )

    def emit_attention(p, qk_t, v_pending=False, bg=None):
        # v_pending: the v projection hasn't been emitted yet; weave one v chain
        # into each group of ig 0 (just before the AV that consumes it) so the
        # exp stream starts immediately instead of idling behind the v proj.
        # bg: next pair's projection chain thunks, spread evenly over this
        # pair's groups so they ride the PE slack instead of bunching.
        bg = list(bg or [])
        ngroups = IG * JT
        start = JT if v_pending else 0
        stride = max(1, (ngroups - start) // max(1, len(bg)))
        gidx = 0
        for ig in range(IG):
            po = [po_pool.tile([65, 512], f32, tag="po", name=f"po{_h}") for _h in range(2)]
            for jj in range(JT):
                ps = pss_pool.tile([128, 1024], f32, tag="pss")
                for h in range(2):
                    base = 64 * h
                    nc.tensor.matmul(
                        ps[:, h * 512 : (h + 1) * 512],
                        qk_t[base : base + 64, 1, jj * 128 : (jj + 1) * 128],
                        qk_t[base : base + 64, 0, ig * 512 : (ig + 1) * 512],
                        start=True,
                        stop=True,
                    )
                e_t = e_pool.tile([128, 1024], bf16, tag="e")
                nc.scalar.activation(e_t, ps, Exp, scale=0.125)
                if v_pending and ig == 0 and jj in v_pending:
                    emit_v_chain(jj)
                if p == 0 and ig == 0 and jj in (5, 10, 15):
                    # own q chains for groups 1-3, emitted before those
                    # groups' scores (emission order defines dependencies)
                    qk_chain(0, qk_t, jj // 5, 0)
                if bg and gidx >= start and (gidx - start) % stride == 0:
                    bg.pop(0)()
                gidx += 1
                for h in range(2):
                    nc.tensor.matmul(
                        po[h],
                        v_sb[:, jj, 2 * p + h, :],
                        e_t[:, h * 512 : (h + 1) * 512],
                        start=(jj == 0),
                        stop=(jj == JT - 1),
                    )
            # normalize: out^T[d, i] = po[d, i] * (1/den[i]) + bv[d], written as out^T.
            # den row sits on partition 64; the reciprocal is computed across 128
            # lanes by bouncing the row through DRAM into a [128, 4] layout, and
            # broadcast back across partitions with a stride-0 DMA.
            o_t = [o_pool.tile([65, 512], f32, tag="o", name=f"ot{_h}") for _h in range(2)]
            rcd_t = dr_pool.tile([2, 512], f32, tag="rcd")
            rct = sm_pool.tile([128, 8], f32, tag="rct")
            for h in range(2):
                nc.vector.tensor_copy(o_t[h], po[h])
                nc.sync.dma_start(rct[:, h * 4 : (h + 1) * 4], o_t[h][64:65, :])
            rcp = sm_pool.tile([128, 8], f32, tag="rcp")
            nc.vector.reciprocal(rcp, rct)
            for h in range(2):
                nc.sync.dma_start(rcd_t[h], rcp[:, h * 4 : (h + 1) * 4])
            for h in range(2):
                den_bc = sm_pool.tile([64, 512], f32, tag="denbc", name=f"dbc{h}")
                rcd_h = rcd_t[h]
                bc_src = bass.AP(
                    tensor=rcd_h.tensor,
                    offset=rcd_h.offset,
                    ap=[[0, 64]] + list(rcd_h.ap),
                )
                nc.gpsimd.dma_start(den_bc, bc_src)
                ot = ot_pool.tile([64, 512], f32, tag="ot")
                nc.vector.tensor_mul(ot, o_t[h][0:64, :], den_bc)
                nc.vector.tensor_scalar_add(
                    ot, ot, bv_pp[:, 2 * p + h : 2 * p + h + 1]
                )
                nc.sync.dma_start(
                    out_d[(2 * p + h) * HS : (2 * p + h + 1) * HS, ig * 512 : (ig + 1) * 512],
                    ot,
                )

    qk_tiles = [qk_pool.tile([128, 2, T], bf16, tag="qkt", name=f"qkt{_p}") for _p in range(PAIRS)]
    # prologue: k chains (needed for all of ig0's scores) interleaved with the
    # first four v chains, which fill the PE idle gaps while x streams in;
    # then q(g0). q(g1..3) and v(4..15) weave into ig0.
    for g in range(IG):
        qk_chain(0, qk_tiles[0], g, 1)
        emit_v_chain(g)
    qk_chain(0, qk_tiles[0], 0, 0)
    import functools
    for p in range(PAIRS):
        bg = []
        if p + 1 < PAIRS:
            bg = [
                functools.partial(qk_chain, p + 1, qk_tiles[p + 1], g, qk)
                for g in range(IG)
                for qk in range(2)
            ]
        emit_attention(
            p, qk_tiles[p],
            v_pending=set(range(4, JT)) if p == 0 else None,
            bg=bg,
        )
    ctx.close()


def _build():
    import concourse.tile as tile
    from concourse import bacc, mybir

    f32 = mybir.dt.float32
    nc = bacc.Bacc("TRN2", target_bir_lowering=False, debug=False, num_devices=8)
    x_d = nc.dram_tensor("x", [D, T], mybir.dt.bfloat16, kind="ExternalInput").ap()
    w_d = nc.dram_tensor("w", [D, WCOLS], mybir.dt.bfloat16, kind="ExternalInput").ap()
    bqk_d = nc.dram_tensor("bqk", [128, PAIRS, 2], f32, kind="ExternalInput").ap()
    bv_d = nc.dram_tensor("bv", [VCOLS], f32, kind="ExternalInput").ap()
    out_d = nc.dram_tensor("out", [VCOLS, T], f32, kind="ExternalOutput").ap()
    with tile.TileContext(nc) as tc:
        _emit(tc, x_d, w_d, bqk_d, bv_d, out_d)
    nc.compile()
    return nc


def get_nc():
    if "nc" not in _CACHE:
        _CACHE["nc"] = _build()
    return _CACHE["nc"]


def make_in_maps(x, W_qkv, b_qkv):
    """Shard full inputs into 8 per-core input maps."""
    x = np.asarray(x, dtype=np.float32)
    W_qkv = np.asarray(W_qkv, dtype=np.float32)
    b_qkv = np.asarray(b_qkv, dtype=np.float32)
    in_maps = []
    for c in range(8):
        b, half = divmod(c, 2)
        w_c = W_qkv[:, half * WCOLS : (half + 1) * WCOLS]
        b_c = b_qkv[half * WCOLS : (half + 1) * WCOLS]
        # permute columns: paired q/k blocks first, then v cols in head order
        w3 = w_c.reshape(D, NH_LOCAL, 3, HS)
        blocks = []
        for p in range(PAIRS):
            for qk in range(2):
                blocks.append(w3[:, 2 * p, qk, :])
                blocks.append(w3[:, 2 * p + 1, qk, :])
        for h in range(NH_LOCAL):
            blocks.append(w3[:, h, 2, :])
        w_c = np.concatenate(blocks, axis=1).astype(ml_dtypes.bfloat16)
        # per-partition qk bias: partitions 0-63 <- head 2p, 64-127 <- head 2p+1
        bqk = np.zeros((128, PAIRS, 2), dtype=np.float32)
        for p in range(PAIRS):
            for qk in range(2):
                bqk[0:64, p, qk] = b_c[(2 * p) * 192 + qk * 64 : (2 * p) * 192 + (qk + 1) * 64]
                bqk[64:128, p, qk] = b_c[(2 * p + 1) * 192 + qk * 64 : (2 * p + 1) * 192 + (qk + 1) * 64]
        bv = np.ascontiguousarray(
            b_c.reshape(NH_LOCAL, 3, HS)[:, 2, :].reshape(VCOLS)
        )
        in_maps.append(
            {
                "x": np.ascontiguousarray(x[b].T).astype(ml_dtypes.bfloat16),
                "w": w_c,
                "bqk": bqk,
                "bv": bv,
            }
        )
    return in_maps


def assemble_output(results):
    out = np.zeros((4, T, D), dtype=np.float32)
    for c in range(8):
        b, half = divmod(c, 2)
        out[b, :, half * VCOLS : (half + 1) * VCOLS] = results[c]["out"].T
    return out


def kernel(x, W_qkv, b_qkv):
    from concourse.bass_utils import run_bass_kernel_spmd

    nc = get_nc()
    in_maps = make_in_maps(x, W_qkv, b_qkv)
    res = run_bass_kernel_spmd(nc, in_maps, core_ids=list(range(8)))
    return assemble_output(res.results)


if __name__ == "__main__":
    xs = np.random.randn(4, T, D).astype(np.float32)
    Ws = (np.random.randn(D, 3 * D) / 32.0).astype(np.float32)
    bs = (np.random.randn(3 * D) * 0.02).astype(np.float32)
    o = kernel(xs, Ws, bs)
    print(o.shape, o.dtype)



# revision 24
# speedup vs baseline: 1.4526x; 1.0068x over previous
"""Fused QKV + multi-head attention kernel for Trainium2 (Bass/Tile), 8-core SPMD.

Problem: x[4, 2048, 1024] -> qkv = x @ W_qkv + b_qkv -> 16-head attention -> out[4, 2048, 1024].

Sharding (DP x TP): core c handles batch c//2 and head-group c%2 (8 of 16 heads),
so each core runs the qkv projection for its batch restricted to its heads'
columns of W_qkv, plus full attention for its 8 heads. No cross-core comm.

Per-core kernel design (bf16 matmuls, fp32 accumulation):
 - the host pre-transposes x during sharding, so the device loads xT[k, tok]
   directly with a casting DMA (f32->bf16); no on-device transposes anywhere.
 - qk projection computes qkv^T directly: lhsT = W tile (layout [k, feat] as
   stored), rhs = xT. Heads are processed in pairs: head 2p lives in SBUF
   partitions 0-63 and head 2p+1 in partitions 64-127 (both q^T and k^T), which
   makes the K=64 score matmuls legal (lhsT/rhs share a base partition) and
   lets the PE run the two heads on independent 64-row array tiles.
 - v projection computes v in [tok, feat] orientation (lhsT = xT stationary,
   rhs = W v-columns), with a ones-column appended per head so the attention
   A@V matmul also produces the softmax denominator.
 - scores^T = k^T.T @ q^T accumulate in PSUM; exp (with the 1/8 scale folded
   into the ACT affine) reads [128, 2048] PSUM spans to amortize ACT overhead;
   no max-subtraction (scores ~ N(0,1), exp never overflows fp32/bf16).
 - out^T[65, 512] = [v | 1].T @ E accumulates over key tiles; the denominator
   row is bounced through DRAM into a [128, 4] layout for a 128-lane reciprocal,
   broadcast back across partitions with a stride-0 DMA, and the normalized
   out^T (+ v-bias, added after normalization -- exact) is stored transposed;
   the host un-transposes during unsharding.
"""

import sys

sys.path.insert(0, "/opt/trn_rl_repo")

import numpy as np
import ml_dtypes

T = 2048
D = 1024
NH_LOCAL = 8  # heads per core
HS = 64
WCOLS = NH_LOCAL * 3 * HS  # 1536
VCOLS = NH_LOCAL * HS  # 512
KT = D // 128  # 8 contraction tiles
PAIRS = NH_LOCAL // 2  # 4
IG = T // 512  # 4 query groups
JT = T // 128  # 16 key tiles

_CACHE: dict = {}


def _emit(tc, x_d, w_d, bqk_d, bv_d, out_d):
    import concourse.bass as bass
    from concourse import mybir
    from contextlib import ExitStack

    nc = tc.nc
    f32 = mybir.dt.float32
    bf16 = mybir.dt.bfloat16
    Exp = mybir.ActivationFunctionType.Exp

    ctx = ExitStack()
    pers = ctx.enter_context(tc.tile_pool(name="pers", bufs=1))
    qk_pool = ctx.enter_context(tc.tile_pool(name="qk", bufs=PAIRS))
    e_pool = ctx.enter_context(tc.tile_pool(name="epool", bufs=3))
    o_pool = ctx.enter_context(tc.tile_pool(name="opool", bufs=3))
    sm_pool = ctx.enter_context(tc.tile_pool(name="smpool", bufs=4))
    ot_pool = ctx.enter_context(tc.tile_pool(name="otpool", bufs=4))
    dr_pool = ctx.enter_context(tc.tile_pool(name="drpool", bufs=4, space="DRAM"))
    # PSUM: 8 banks total = ps 2 (proj) + pss 2x2 (scores/exp staging) + po 2 (AV accum).
    # pss double-buffering is what keeps the scores->exp->AV pipeline from
    # serializing (a single buffer stalls the PE ~2.7us per exp group, which
    # also re-throttles the HAM clock gate to 1.2 GHz).
    ps_pool = ctx.enter_context(tc.tile_pool(name="pspool", bufs=2, space="PSUM"))
    pss_pool = ctx.enter_context(tc.tile_pool(name="psspool", bufs=2, space="PSUM"))
    po_pool = ctx.enter_context(tc.tile_pool(name="popool", bufs=2, space="PSUM"))

    # ---- load xT (host pre-transposes and pre-casts to bf16) ----
    xt_sb = pers.tile([128, KT, T], bf16)  # xT[k, tok] per k-tile
    for kk in range(KT):
        nc.sync.dma_start(xt_sb[:, kk, :], x_d[kk * 128 : (kk + 1) * 128, :])

    # ---- constants ----
    bqk_sb = pers.tile([128, PAIRS, 2], f32)  # [part, pair, q/k] per-partition bias
    nc.sync.dma_start(bqk_sb, bqk_d)
    bv_pp = pers.tile([64, NH_LOCAL], f32)  # v-bias, per-partition layout [d, head]
    nc.sync.dma_start(bv_pp, bv_d.rearrange("(h d) -> d h", d=HS))

    # ---- load W (host pre-casts to bf16); pair-0 qk columns first ----
    w_sb = pers.tile([128, KT, WCOLS], bf16)
    for kk in range(KT):
        nc.sync.dma_start(w_sb[:, kk, 0:256], w_d[kk * 128 : (kk + 1) * 128, 0:256])
    for kk in range(KT):
        nc.sync.dma_start(
            w_sb[:, kk, 256:WCOLS], w_d[kk * 128 : (kk + 1) * 128, 256:WCOLS]
        )

    # ---- v ones-columns ----
    v_sb = pers.tile([128, JT, NH_LOCAL, HS + 1], bf16)
    nc.vector.memset(v_sb[:, :, :, HS : HS + 1], 1.0)

    # w is host-permuted: cols [(p*2+qk)*128 : +128] = paired q/k block for pair p,
    # cols [1024:1536] = v columns in head order (walrus requires single-free-dim
    # matmul operand APs, so the pairing permutation happens host-side).
    def qk_chain(p, qk_t, g, qk):
        # one projection chain: q^T or k^T for pair p, token group g
        ps = ps_pool.tile([128, 512], f32, tag="ps", name="psqk")
        c0 = (p * 2 + qk) * 128
        for kk in range(KT):
            nc.tensor.matmul(
                ps,
                w_sb[:, kk, c0 : c0 + 128],
                xt_sb[:, kk, g * 512 : (g + 1) * 512],
                start=(kk == 0),
                stop=(kk == KT - 1),
            )
        nc.vector.tensor_scalar_add(
            qk_t[:, qk, g * 512 : (g + 1) * 512], ps, bqk_sb[:, p, qk : qk + 1]
        )

    def emit_qk_proj(p, qk_t):
        # psum partitions 0-63 <- head 2p, 64-127 <- head 2p+1
        for g in range(IG):
            for qk in range(2):
                qk_chain(p, qk_t, g, qk)

    def emit_v_chain(tt):
        # one v-projection chain: v[tok tile tt, all heads] = xT.T @ Wv
        ps = ps_pool.tile([128, 512], f32, tag="ps", name="psv")
        for kk in range(KT):
            nc.tensor.matmul(
                ps,
                xt_sb[:, kk, tt * 128 : (tt + 1) * 128],
                w_sb[:, kk, 1024:1536],
                start=(kk == 0),
                stop=(kk == KT - 1),
            )
        nc.vector.tensor_copy(
            v_sb[:, tt, :, 0:HS], ps.rearrange("p (h c) -> p h c", c=HS)
        )

    def emit_attention(p, qk_t, v_pending=False, bg=None):
        # v_pending: the v projection hasn't been emitted yet; weave one v chain
        # into each group of ig 0 (just before the AV that consumes it) so the
        # exp stream starts immediately instead of idling behind the v proj.
        # bg: next pair's projection chain thunks, spread evenly over this
        # pair's groups so they ride the PE slack instead of bunching.
        bg = list(bg or [])
        ngroups = IG * JT
        start = JT if v_pending else 0
        stride = max(1, (ngroups - start) // max(1, len(bg)))
        gidx = 0
        for ig in range(IG):
            po = [po_pool.tile([65, 512], f32, tag="po", name=f"po{_h}") for _h in range(2)]
            for jj in range(JT):
                ps = pss_pool.tile([128, 1024], f32, tag="pss")
                for h in range(2):
                    base = 64 * h
                    nc.tensor.matmul(
                        ps[:, h * 512 : (h + 1) * 512],
                        qk_t[base : base + 64, 1, jj * 128 : (jj + 1) * 128],
                        qk_t[base : base + 64, 0, ig * 512 : (ig + 1) * 512],
                        start=True,
                        stop=True,
                    )
                e_t = e_pool.tile([128, 1024], bf16, tag="e")
                nc.scalar.activation(e_t, ps, Exp, scale=0.125)
                if v_pending and ig == 0:
                    emit_v_chain(jj)
                if bg and gidx >= start and (gidx - start) % stride == 0:
                    bg.pop(0)()
                gidx += 1
                for h in range(2):
                    nc.tensor.matmul(
                        po[h],
                        v_sb[:, jj, 2 * p + h, :],
                        e_t[:, h * 512 : (h + 1) * 512],
                        start=(jj == 0),
                        stop=(jj == JT - 1),
                    )
            # normalize: out^T[d, i] = po[d, i] * (1/den[i]) + bv[d], written as out^T.
            # den row sits on partition 64; the reciprocal is computed across 128
            # lanes by bouncing the row through DRAM into a [128, 4] layout, and
            # broadcast back across partitions with a stride-0 DMA.
            o_t = [o_pool.tile([65, 512], f32, tag="o", name=f"ot{_h}") for _h in range(2)]
            rcd_t = dr_pool.tile([2, 512], f32, tag="rcd")
            rct = sm_pool.tile([128, 8], f32, tag="rct")
            for h in range(2):
                nc.vector.tensor_copy(o_t[h], po[h])
                nc.sync.dma_start(rct[:, h * 4 : (h + 1) * 4], o_t[h][64:65, :])
            rcp = sm_pool.tile([128, 8], f32, tag="rcp")
            nc.vector.reciprocal(rcp, rct)
            for h in range(2):
                nc.sync.dma_start(rcd_t[h], rcp[:, h * 4 : (h + 1) * 4])
            for h in range(2):
                den_bc = sm_pool.tile([64, 512], f32, tag="denbc", name=f"dbc{h}")
                rcd_h = rcd_t[h]
                bc_src = bass.AP(
                    tensor=rcd_h.tensor,
                    offset=rcd_h.offset,
                    ap=[[0, 64]] + list(rcd_h.ap),
                )
                nc.gpsimd.dma_start(den_bc, bc_src)
                ot = ot_pool.tile([64, 512], f32, tag="ot")
                nc.vector.tensor_mul(ot, o_t[h][0:64, :], den_bc)
                nc.vector.tensor_scalar_add(
                    ot, ot, bv_pp[:, 2 * p + h : 2 * p + h + 1]
                )
                nc.sync.dma_start(
                    out_d[(2 * p + h) * HS : (2 * p + h + 1) * HS, ig * 512 : (ig + 1) * 512],
                    ot,
                )

    qk_tiles = [qk_pool.tile([128, 2, T], bf16, tag="qkt", name=f"qkt{_p}") for _p in range(PAIRS)]
    emit_qk_proj(0, qk_tiles[0])
    import functools
    for p in range(PAIRS):
        bg = []
        if p + 1 < PAIRS:
            bg = [
                functools.partial(qk_chain, p + 1, qk_tiles[p + 1], g, qk)
                for g in range(IG)
                for qk in range(2)
            ]
        emit_attention(p, qk_tiles[p], v_pending=(p == 0), bg=bg)
        for fn in bg:
            pass
    ctx.close()


def _build():
    import concourse.tile as tile
    from concourse import bacc, mybir

    f32 = mybir.dt.float32
    nc = bacc.Bacc("TRN2", target_bir_lowering=False, debug=False, num_devices=8)
    x_d = nc.dram_tensor("x", [D, T], mybir.dt.bfloat16, kind="ExternalInput").ap()
    w_d = nc.dram_tensor("w", [D, WCOLS], mybir.dt.bfloat16, kind="ExternalInput").ap()
    bqk_d = nc.dram_tensor("bqk", [128, PAIRS, 2], f32, kind="ExternalInput").ap()
    bv_d = nc.dram_tensor("bv", [VCOLS], f32, kind="ExternalInput").ap()
    out_d = nc.dram_tensor("out", [VCOLS, T], f32, kind="ExternalOutput").ap()
    with tile.TileContext(nc) as tc:
        _emit(tc, x_d, w_d, bqk_d, bv_d, out_d)
    nc.compile()
    return nc


def get_nc():
    if "nc" not in _CACHE:
        _CACHE["nc"] = _build()
    return _CACHE["nc"]


def make_in_maps(x, W_qkv, b_qkv):
    """Shard full inputs into 8 per-core input maps."""
    x = np.asarray(x, dtype=np.float32)
    W_qkv = np.asarray(W_qkv, dtype=np.float32)
    b_qkv = np.asarray(b_qkv, dtype=np.float32)
    in_maps = []
    for c in range(8):
        b, half = divmod(c, 2)
        w_c = W_qkv[:, half * WCOLS : (half + 1) * WCOLS]
        b_c = b_qkv[half * WCOLS : (half + 1) * WCOLS]
        # permute columns: paired q/k blocks first, then v cols in head order
        w3 = w_c.reshape(D, NH_LOCAL, 3, HS)
        blocks = []
        for p in range(PAIRS):
            for qk in range(2):
                blocks.append(w3[:, 2 * p, qk, :])
                blocks.append(w3[:, 2 * p + 1, qk, :])
        for h in range(NH_LOCAL):
            blocks.append(w3[:, h, 2, :])
        w_c = np.concatenate(blocks, axis=1).astype(ml_dtypes.bfloat16)
        # per-partition qk bias: partitions 0-63 <- head 2p, 64-127 <- head 2p+1
        bqk = np.zeros((128, PAIRS, 2), dtype=np.float32)
        for p in range(PAIRS):
            for qk in range(2):
                bqk[0:64, p, qk] = b_c[(2 * p) * 192 + qk * 64 : (2 * p) * 192 + (qk + 1) * 64]
                bqk[64:128, p, qk] = b_c[(2 * p + 1) * 192 + qk * 64 : (2 * p + 1) * 192 + (qk + 1) * 64]
        bv = np.ascontiguousarray(
            b_c.reshape(NH_LOCAL, 3, HS)[:, 2, :].reshape(VCOLS)
        )
        in_maps.append(
            {
                "x": np.ascontiguousarray(x[b].T).astype(ml_dtypes.bfloat16),
                "w": w_c,
                "bqk": bqk,
                "bv": bv,
            }
        )
    return in_maps


def assemble_output(results):
    out = np.zeros((4, T, D), dtype=np.float32)
    for c in range(8):
        b, half = divmod(c, 2)
        out[b, :, half * VCOLS : (half + 1) * VCOLS] = results[c]["out"].T
    return out


def kernel(x, W_qkv, b_qkv):
    from concourse.bass_utils import run_bass_kernel_spmd

    nc = get_nc()
    in_maps = make_in_maps(x, W_qkv, b_qkv)
    res = run_bass_kernel_spmd(nc, in_maps, core_ids=list(range(8)))
    return assemble_output(res.results)


if __name__ == "__main__":
    xs = np.random.randn(4, T, D).astype(np.float32)
    Ws = (np.random.randn(D, 3 * D) / 32.0).astype(np.float32)
    bs = (np.random.randn(3 * D) * 0.02).astype(np.float32)
    o = kernel(xs, Ws, bs)
    print(o.shape, o.dtype)

